# revision 67
# baseline (speedup 1.0000x reference)
"""Grouped-query attention, tensor-parallel over heads across 8 TRN2 NeuronCores.

Problem (hardcoded): x[1,1024,4096] @ Wq/Wk/Wv -> RoPE -> causal GQA
(32 q heads, 8 kv groups, head_dim 128) -> out proj Wo -> [1,1024,4096].

Sharding: core r owns q heads 4r..4r+3 and kv group r (Wq/Wk/Wv column
shards, Wo row shard). Each core computes a full [1024,4096] partial of
the output projection; the host sums the 8 partials (the "all-reduce").

Device kernel (per core): the big GEMMs (Q/K/V projections, out-proj)
run in fp8e4 DoubleRow mode, which processes two 128-deep contraction
chunks per instruction at half the cycles/row of bf16.  Precision is
recovered with a 3-term hi/lo split quantization (x_hi@W_hi + x_lo@W_hi
+ x_hi@W_lo), where hi and lo shares one power-of-2 scale so all terms
accumulate in a single PSUM chain; measured end-to-end error matches
bf16.  The attention core (scores, exp, ctx) stays bf16 with 256-wide
s-blocks and causal tile skipping.  The softmax denominator is computed
off the tensor engine entirely: a DVE tree-sum of the P tiles, a Pool
partition_all_reduce, and a bf16 reciprocal.  The first s-block's
attention stages are pre-emitted into the second projection half's
instruction stream (scores matmuls fill DMA-paced slack; their exps run
on the then-idle Act engine), and output-projection tiles stream out
through partial-region DMAs so only a 512-column transfer trails the
final matmul.
"""

import numpy as np
import ml_dtypes

import concourse.bass as bass
import concourse.bacc as bacc
import concourse.mybir as mybir
import concourse.tile as tile
import concourse.bass_isa as bass_isa
from concourse.bass_utils import run_bass_kernel_spmd

S = 1024          # sequence length
D = 4096          # model dim
H = 32            # query heads (global)
G = 8             # kv groups (global)
HD = 128          # head dim
N_CORES = 8
HPC = H // N_CORES   # 4 query heads per core
QW = HPC * HD        # 512 q-proj cols per core
NDC = D // 128       # 32 contraction chunks
NP = NDC // 2        # 16 DoubleRow chunk pairs
BF = mybir.dt.bfloat16
F8 = mybir.dt.float8e4
F32 = mybir.dt.float32
DR = mybir.MatmulPerfMode.DoubleRow

# quantization scales (powers of 2; hi and lo share the scale so every
# 3-term matmul accumulates in one PSUM chain)
XS = 16.0
WQS = 8192.0        # applied to Wq/sqrt(HD)
WKS = 1024.0
WVS = 1024.0
WOS = 1024.0
CTXS = 16.0
EXP_SHIFT = -6.0    # exp(s - 6): keeps bf16 P comfortably in range
N_WARM = 38         # startup PE warm-up matmuls bridging the DMA fill

_CACHE = {}


def _build():
    nc = bacc.Bacc("TRN2", target_bir_lowering=False, debug=False,
                   num_devices=N_CORES)

    xb = nc.dram_tensor("xb", [128, NDC, 2, S], F8, kind="ExternalInput")
    wqb = nc.dram_tensor("wqb", [128, NDC, 2, QW], F8, kind="ExternalInput")
    wkb = nc.dram_tensor("wkb", [128, NDC, 2, HD], F8, kind="ExternalInput")
    wvb = nc.dram_tensor("wvb", [128, NDC, 2, HD], F8, kind="ExternalInput")
    wob = nc.dram_tensor("wob", [128, 2, HPC, D], F8, kind="ExternalInput")
    cosT = nc.dram_tensor("cosT", [HD, S], BF, kind="ExternalInput")
    sinT = nc.dram_tensor("sinT", [HD, S], BF, kind="ExternalInput")
    rmat = nc.dram_tensor("rmat", [HD, HD], BF, kind="ExternalInput")
    masks = nc.dram_tensor("masks", [128, 512], BF, kind="ExternalInput")
    out = nc.dram_tensor("out", [S, D], BF, kind="ExternalOutput")

    with tile.TileContext(nc) as tc:
        _emit(tc, nc, xb, wqb, wkb, wvb, wob,
              cosT, sinT, rmat, masks, out)
    nc.compile()
    return nc


def _emit(tc, nc, xb, wqb, wkb, wvb, wob,
          cosT, sinT, rmat, masks, out):
    import contextlib
    ctx = contextlib.ExitStack()
    with ctx:
        const = ctx.enter_context(tc.tile_pool(name="const", bufs=1))
        work = ctx.enter_context(tc.tile_pool(name="work", bufs=1))
        tmp = ctx.enter_context(tc.tile_pool(name="tmp", bufs=4))
        pt_pool = ctx.enter_context(tc.tile_pool(name="pt", bufs=8))
        outp = ctx.enter_context(tc.tile_pool(name="outp", bufs=2))
        ps = ctx.enter_context(tc.tile_pool(name="ps", bufs=8, space="PSUM"))

        # ---- DMA emission, ordered to pace the chunk-major PE stream ----
        rmat_sb = const.tile([HD, HD], BF, tag="rmat")
        # rmat memset first: it gates the PE warm-up start
        nc.vector.memset(rmat_sb[:], 0.5)
        ebias = const.tile([128, 1], F32, tag="ebias")
        nc.gpsimd.memset(ebias[:], EXP_SHIFT)

        wk_sb = const.tile([128, NDC, 2, HD], F8, tag="wk")   # dim2: hi/lo

        # PE warm-up bridge: junk matmuls over the startup DMA fill keep the
        # tensor engine's clock-ramp streak alive until real operands land.
        wps = ps.tile([128, 512], F32, tag="ps", name="wps")
        for wi in range(N_WARM):
            nc.tensor.matmul(wps[:, 0:HD], rmat_sb[:], rmat_sb[:],
                             start=(wi == 0), stop=(wi == N_WARM - 1))
        nc.vector.tensor_copy(rmat_sb[:], wps[:, 0:HD])
        nc.sync.dma_start(out=rmat_sb[:], in_=rmat.ap())

        gx4, gq4 = {}, {}
        for c in range(0, NDC, 4):
            gx4[c] = const.tile([128, 4, 2, S], F8, tag=f"x{c//4}", name=f"x{c//4}")
        gxh = {c: gx4[c - c % 4][:, c % 4:c % 4 + 2, 0, :] for c in range(0, NDC, 2)}
        gxl = {c: gx4[c - c % 4][:, c % 4:c % 4 + 2, 1, :] for c in range(0, NDC, 2)}
        gqh, gql = {}, {}
        # half-0 of x, wq, and wk interleaved in consumption order
        for c in range(0, NDC, 4):
            nc.sync.dma_start(out=wk_sb[:, c:c + 4, :, :],
                              in_=wkb.ap()[:, c:c + 4, :, :])
            g = const.tile([128, 4, 2, QW], F8, tag=f"q{c//4}", name=f"q{c//4}")
            gqh[c], gqh[c + 2] = g[:, 0:2, 0, :], g[:, 2:4, 0, :]
            gql[c], gql[c + 2] = g[:, 0:2, 1, :], g[:, 2:4, 1, :]
            if c == 0:
                # fine-grained early groups so the PE stream starts early
                for c2 in (0, 2):
                    nc.sync.dma_start(out=g[:, c2:c2 + 2, :, :],
                                      in_=wqb.ap()[:, c + c2:c + c2 + 2, :, :])
                    nc.sync.dma_start(
                        out=gx4[c][:, c2:c2 + 2, :, 0:512],
                        in_=xb.ap()[:, c + c2:c + c2 + 2, :, 0:512])
            else:
                for c2 in (0, 2):
                    nc.sync.dma_start(out=g[:, c2:c2 + 2, :, :],
                                      in_=wqb.ap()[:, c + c2:c + c2 + 2, :, :])
                    nc.sync.dma_start(
                        out=gx4[c][:, c2:c2 + 2, :, 0:512],
                        in_=xb.ap()[:, c + c2:c + c2 + 2, :, 0:512])
        cos_sb = const.tile([HD, S], BF, tag="cos")
        nc.sync.dma_start(out=cos_sb[:], in_=cosT.ap())
        sin_sb = const.tile([HD, S], BF, tag="sin")
        nc.sync.dma_start(out=sin_sb[:], in_=sinT.ap())
        # half-1 of x
        for c in range(0, NDC, 4):
            nc.sync.dma_start(out=gx4[c][:, :, :, 512:S],
                              in_=xb.ap()[:, c:c + 4, :, 512:S])
        wv_sb = const.tile([128, NDC, 2, HD], F8, tag="wv")
        nc.sync.dma_start(out=wv_sb[:], in_=wvb.ap())
        mask_sb = const.tile([128, 512], BF, tag="mask")
        nc.sync.dma_start(out=mask_sb[:], in_=masks.ap())
        wo_sb = const.tile([128, 2, HPC, D], F8, tag="wo")    # dim1: hi/lo
        for n in range(2):
            sl = slice(n * 2048, (n + 1) * 2048)
            nc.sync.dma_start(out=wo_sb[:, :, :, sl], in_=wob.ap()[:, :, :, sl])

        # persistent activations
        khat = work.tile([HD, S], BF, tag="khat")
        qhat = [work.tile([HD, S], BF, tag=f"qhat{h}", name=f"qhat{h}")
                for h in range(HPC)]
        v_sb = [work.tile([128, HD], BF, tag=f"v{i}", name=f"v{i}")
                for i in range(8)]
        ctx_hi = [work.tile([128, 2, S], F8, tag=f"cth{u}", name=f"cth{u}")
                  for u in range(2)]
        ctx_lo = [work.tile([128, 2, S], F8, tag=f"ctl{u}", name=f"ctl{u}")
                  for u in range(2)]

        # ---- K+Q projections: chunk-major across 5 chains per s-quarter ----
        # Per chunk pair, all five tensors advance their 3-term DoubleRow
        # chains, so the PE stream follows the x/wq DMA arrival order.
        # RoPE for each finished s-half is queued and its PE/Act/DVE work is
        # injected into later quarters' streams (and the v-projection).
        TENS = [("k", khat, lambda c: wk_sb[:, c:c + 2, 0, :],
                 lambda c: wk_sb[:, c:c + 2, 1, :], 1.0 / (XS * WKS))]
        for h in range(HPC):
            hsl = slice(h * HD, (h + 1) * HD)
            TENS.append((f"q{h}", qhat[h],
                         lambda c, s=hsl: gqh[c][:, :, s],
                         lambda c, s=hsl: gql[c][:, :, s], 1.0 / (XS * WQS)))
        raws = {ti: work.tile([HD, S], BF, tag=f"raw{ti}", name=f"raw{ti}")
                for ti in range(5)}

        pend = []   # queued injection closures (ropes, early scores)

        def inject_rope():
            if pend:
                pend.pop(0)()

        def rope_half(ti, half):
            name, dst, _, _, _ = TENS[ti]
            sl = slice(half * 512, (half + 1) * 512)
            t1 = tmp.tile([HD, 512], BF, tag="rope_t1", name="rope_t1", bufs=1)
            nc.vector.tensor_mul(t1[:], raws[ti][:, sl], cos_sb[:, sl])
            rq = ps.tile([HD, 512], F32, tag="ps", name="rq")
            nc.tensor.matmul(rq[:], rmat_sb[:], raws[ti][:, sl],
                             start=True, stop=True)
            t2 = tmp.tile([HD, 512], BF, tag="rope_t2", name="rope_t2", bufs=1)
            nc.vector.tensor_mul(t2[:], rq[:], sin_sb[:, sl])
            nc.vector.tensor_add(dst[:, sl], t1[:], t2[:])

        def v_terms(vpsum, i, p):
            c = 2 * p
            tsl = slice(i * 128, (i + 1) * 128)
            nc.tensor.matmul(vpsum[:], gxh[c][:, :, tsl], wv_sb[:, c:c + 2, 0, :],
                             start=(p == 0), stop=False,
                             perf_mode=DR, skip_group_check=True)
            nc.tensor.matmul(vpsum[:], gxh[c][:, :, tsl], wv_sb[:, c:c + 2, 1, :],
                             start=False, stop=False,
                             perf_mode=DR, skip_group_check=True)
            nc.tensor.matmul(vpsum[:], gxl[c][:, :, tsl], wv_sb[:, c:c + 2, 0, :],
                             start=False, stop=(p == NP - 1),
                             perf_mode=DR, skip_group_check=True)

        pre_pts = {}   # stages pre-emitted into the QK stream
        def emit_scores(h, b):
            ssl = slice(b * 256, (b + 1) * 256)
            pts = []
            for tp in range(b + 1):
                st = ps.tile([128, 512], F32, tag="ps", name="st")
                for i in range(2):
                    t0 = (2 * tp + i) * 128
                    nc.tensor.matmul(st[:, i * 256:(i + 1) * 256],
                                     khat[:, t0:t0 + 128], qhat[h][:, ssl],
                                     start=True, stop=True)
                pt = pt_pool.tile([128, 512], BF, tag="pt", name="pt")
                nc.scalar.activation(pt[:], st[:],
                                     mybir.ActivationFunctionType.Exp,
                                     bias=ebias[:])
                if tp == b:
                    nc.vector.tensor_mul(pt[:], pt[:], mask_sb[:])
                pts.append(pt)
            # Denominator off the PE: DVE tree-sum of P tiles, Pool
            # partition-all-reduce, DVE half-merge + reciprocal. All emitted
            # here so the result has a full pipeline stage of slack.
            if b == 0:
                acc = pts[0]
            else:
                acc = tmp.tile([128, 512], BF, tag="dacc", name="dacc", bufs=2)
                if b == 1:
                    nc.vector.tensor_add(acc[:], pts[0][:], pts[1][:])
                elif b == 2:
                    t01 = tmp.tile([128, 512], BF, tag="dacc2", name="dacc2",
                                   bufs=1)
                    nc.vector.tensor_add(t01[:], pts[0][:], pts[1][:])
                    nc.vector.tensor_add(acc[:], t01[:], pts[2][:])
                else:
                    t01 = tmp.tile([128, 512], BF, tag="dacc2", name="dacc2",
                                   bufs=1)
                    t23 = tmp.tile([128, 512], BF, tag="dacc3", name="dacc3",
                                   bufs=1)
                    nc.vector.tensor_add(t01[:], pts[0][:], pts[1][:])
                    nc.vector.tensor_add(t23[:], pts[2][:], pts[3][:])
                    nc.vector.tensor_add(acc[:], t01[:], t23[:])
            denb = tmp.tile([128, 512], BF, tag="denb", name="denb", bufs=2)
            nc.gpsimd.partition_all_reduce(denb[:], acc[:], 128,
                                           bass_isa.ReduceOp.add)
            rec = tmp.tile([128, 256], BF, tag="rec", name="rec", bufs=5)
            with nc.allow_low_precision(reason="den scale, 0.2% rel is fine"):
                nc.vector.tensor_add(rec[:], denb[:, 0:256],
                                     denb[:, 256:512])
                nc.vector.reciprocal(rec[:], rec[:])
            return pts, rec


        v_first = []   # first v-chains, interleaved with half-1 descales

        for half in (0, 1):
            hoff = half * 512
            chains = [ps.tile([128, 512], F32, tag="ps", name=f"ch{ti}")
                      for ti in range(5)]
            for p in range(NP):
                c = 2 * p
                for term in range(3):
                    for ti, (_, _, whi, wlo, _) in enumerate(TENS):
                        pp = chains[ti]
                        w = whi(c) if term != 1 else wlo(c)
                        for q in range(2):
                            ssl = slice(hoff + q * 256, hoff + (q + 1) * 256)
                            osl = slice(q * 256, (q + 1) * 256)
                            xop = gxl[c] if term == 2 else gxh[c]
                            nc.tensor.matmul(
                                pp[:, osl], w, xop[:, :, ssl],
                                start=(p == 0 and term == 0 and q == 0),
                                stop=(p == NP - 1 and term == 2 and q == 1),
                                perf_mode=DR, skip_group_check=True)
                if p in (2, 4, 6, 8, 10, 12, 14):
                    inject_rope()
            if half == 0:
                for ti, (_, _, _, _, descale) in enumerate(TENS):
                    nc.scalar.activation(raws[ti][:, 0:512], chains[ti][:],
                                         mybir.ActivationFunctionType.Copy,
                                         scale=descale)
                def early_scores(h):
                    pre_pts[(h, 0)] = emit_scores(h, 0)
                pend.extend([
                    lambda: rope_half(0, 0),    # k
                    lambda: rope_half(1, 0),    # q0
                    lambda: rope_half(2, 0),    # q1
                    lambda: early_scores(0),
                    lambda: rope_half(3, 0),    # q2
                    lambda: early_scores(1),
                    lambda: rope_half(4, 0),    # q3
                ])
            else:
                # defer the 5 serial ACT descales until after the first
                # attention stage's exps are queued (ACT priority)
                def half1_tail(chains=chains):
                    for ti, (_, _, _, _, descale) in enumerate(TENS):
                        nc.scalar.activation(
                            raws[ti][:, 512:S], chains[ti][:],
                            mybir.ActivationFunctionType.Copy, scale=descale)
                        pend.append(lambda t=ti: rope_half(t, 1))
                v_first.append(half1_tail)

        # ---- V projection: emitted as PE filler inside early attention ----
        def v_chain(i):
            tsl = slice(i * 128, (i + 1) * 128)
            vp = ps.tile([128, HD], F32, tag="ps", name="vp")
            for p in range(NP):
                c = 2 * p
                nc.tensor.matmul(vp[:], gxh[c][:, :, tsl], wv_sb[:, c:c + 2, 0, :],
                                 start=(p == 0), stop=False, perf_mode=DR)
            for p in range(NP):
                c = 2 * p
                nc.tensor.matmul(vp[:], gxh[c][:, :, tsl], wv_sb[:, c:c + 2, 1, :],
                                 start=False, stop=False, perf_mode=DR)
            for p in range(NP):
                c = 2 * p
                nc.tensor.matmul(vp[:], gxl[c][:, :, tsl], wv_sb[:, c:c + 2, 0, :],
                                 start=False, stop=(p == NP - 1), perf_mode=DR)
            nc.scalar.activation(v_sb[i][:], vp[:],
                                 mybir.ActivationFunctionType.Copy,
                                 scale=1.0 / (XS * WVS))
            inject_rope()

        pre_pts[(2, 0)] = emit_scores(2, 0)
        v_first[0]()          # the deferred half-1 descales
        v_chain(0)
        pre_pts[(3, 0)] = emit_scores(3, 0)
        v_chain(1)
        vq = list(range(2, 8))

        def emit_denctx(h, b, pts, rec):
            ssl = slice(b * 256, (b + 1) * 256)
            cx = ps.tile([HD, 256], F32, tag="ps", name="cx")
            n_mm = 2 * (b + 1)
            k = 0
            for tp, pt in enumerate(pts):
                for i in range(2):
                    if tp == b and i == 1:
                        # diagonal pair: tokens 128-255 only reach s>=128
                        nc.tensor.matmul(cx[:, 128:256], v_sb[2 * tp + i][:],
                                         pt[:, 384:512],
                                         start=False, stop=(k == n_mm - 1))
                    else:
                        nc.tensor.matmul(cx[:], v_sb[2 * tp + i][:],
                                         pt[:, i * 256:(i + 1) * 256],
                                         start=(k == 0), stop=(k == n_mm - 1))
                    k += 1
            ctxn = tmp.tile([HD, 256], F32, tag="ctxn", name="ctxn", bufs=1)
            nc.vector.scalar_tensor_tensor(
                ctxn[:], cx[:], CTXS, rec[:],
                op0=mybir.AluOpType.mult, op1=mybir.AluOpType.mult)
            u, par = divmod(h, 2)
            nc.vector.tensor_copy(ctx_hi[u][:, par, ssl], ctxn[:])
            nc.vector.tensor_sub(ctx_lo[u][:, par, ssl], ctxn[:],
                                 ctx_hi[u][:, par, ssl])

        descale = 1.0 / (CTXS * WOS)

        def emit_outproj(t8, last=False):
            tsl = slice(t8 * 128, (t8 + 1) * 128)
            for n2 in range(2):
                ot = outp.tile([128, 2048], BF, tag="ot", name="ot")
                # partial-region DMAs [1024|512|512]; the very last tile
                # uses [1536|512] so only one HWDGE generation trails the
                # final matmul
                if last and n2 == 1:
                    dma_after = {1: (0, 1024), 3: (1024, 1024)}
                else:
                    dma_after = {1: (0, 1024), 2: (1024, 512),
                                 3: (1536, 512)}
                for half2 in range(4):
                    op = ps.tile([128, 512], F32, tag="ps", name="op")
                    for sub2 in range(2):
                        n = 8 * n2 + 2 * half2 + sub2
                        nsl = slice(n * 256, (n + 1) * 256)
                        osl = slice(sub2 * 256, (sub2 + 1) * 256)
                        k = 0
                        for u in range(2):
                            for chi, whi in ((ctx_hi, 0), (ctx_lo, 0),
                                             (ctx_hi, 1)):
                                nc.tensor.matmul(
                                    op[:, osl], chi[u][:, :, tsl],
                                    wo_sb[:, whi, 2 * u:2 * u + 2, nsl],
                                    start=(k == 0 and sub2 == 0),
                                    stop=(k == 5 and sub2 == 1),
                                    perf_mode=DR, skip_group_check=True)
                                k += 1
                    osl2 = slice(half2 * 512, (half2 + 1) * 512)
                    if half2 % 2 == 0:
                        nc.vector.tensor_scalar_mul(ot[:, osl2], op[:],
                                                    descale)
                    else:
                        nc.scalar.activation(
                            ot[:, osl2], op[:],
                            mybir.ActivationFunctionType.Copy,
                            scale=descale)
                    if half2 in dma_after:
                        o0, ow = dma_after[half2]
                        nc.sync.dma_start(
                            out=out.ap()[tsl, n2 * 2048 + o0:
                                         n2 * 2048 + o0 + ow],
                            in_=ot[:, o0:o0 + ow])

        stages = [(h, b) for b in range(4) for h in range(HPC)]
        prev = None
        outq = []
        cooldown = 0
        for si, hb in enumerate(stages):
            if hb in pre_pts:
                pts, dacc = pre_pts.pop(hb)
            else:
                pts, dacc = emit_scores(*hb)
            todo = None
            if cooldown:
                cooldown -= 1
            elif outq:
                todo = outq.pop(0)
            if prev is not None:
                (ph, pb), ppts, pdacc = prev
                emit_denctx(ph, pb, ppts, pdacc)
                if ph == HPC - 1:
                    outq.extend([2 * pb + 1, 2 * pb])
                    cooldown = 1
            if todo is not None:
                emit_outproj(todo)
            inject_rope()
            for _ in range(2 if si < 3 else 1):
                if vq:
                    v_chain(vq.pop(0))
            if si + 1 < len(stages):
                nxt = stages[si + 1]
                if nxt not in pre_pts:
                    pre_pts[nxt] = emit_scores(*nxt)
            prev = (hb, pts, dacc)
        (ph, pb), ppts, pdacc = prev
        emit_denctx(ph, pb, ppts, pdacc)
        outq.extend([2 * pb + 1, 2 * pb])
        for qi, t8 in enumerate(outq):
            emit_outproj(t8, last=(qi == len(outq) - 1))


def _prep_inputs(x, cos, sin, Wq, Wk, Wv, Wo):
    """Host-side shard + hi/lo fp8 quantization. Returns per-core inputs."""
    bf = ml_dtypes.bfloat16
    f8 = ml_dtypes.float8_e4m3

    def hilo(a, s):
        hi = np.asarray(a * s, np.float32).astype(f8)
        lo = (np.asarray(a * s, np.float32) - hi.astype(np.float32)).astype(f8)
        return hi, lo

    x2 = np.asarray(x, np.float32).reshape(S, D)
    xTh = np.ascontiguousarray(x2.T).reshape(NDC, 128, S).transpose(1, 0, 2)
    xh_, xl_ = hilo(np.ascontiguousarray(xTh), XS)
    xb_ = np.ascontiguousarray(np.stack([xh_, xl_], axis=2))  # [128,NDC,2,S]

    cosT = np.ascontiguousarray(np.asarray(cos, np.float32).T).astype(bf)
    sinT = np.ascontiguousarray(np.asarray(sin, np.float32).T).astype(bf)

    rmat = np.zeros((HD, HD), np.float32)
    half = HD // 2
    rmat[np.arange(half), np.arange(half) + half] = 1.0
    rmat[np.arange(half) + half, np.arange(half)] = -1.0
    rmat = rmat.astype(bf)

    # diagonal pair mask: keep when t_local (= i*128 + p) <= s_local
    lt = np.arange(128)[:, None]
    ls = np.arange(256)[None, :]
    masks = np.concatenate([(lt + 128 * i <= ls) for i in range(2)], axis=1)
    masks = np.ascontiguousarray(masks).astype(bf)     # [128, 512]

    scale = 1.0 / np.sqrt(np.float32(HD))
    Wq_ = np.asarray(Wq, np.float32) * scale
    Wk_ = np.asarray(Wk, np.float32)
    Wv_ = np.asarray(Wv, np.float32)
    Wo_ = np.asarray(Wo, np.float32)

    def chunked(w):  # [D, m] -> [128, NDC, m]
        m = w.shape[1]
        return np.ascontiguousarray(
            w.reshape(NDC, 128, m).transpose(1, 0, 2))

    in_maps = []
    for r in range(N_CORES):
        wqh_, wql_ = hilo(chunked(Wq_[:, r * QW:(r + 1) * QW]), WQS)
        wqb_ = np.ascontiguousarray(np.stack([wqh_, wql_], axis=2))
        wkh_, wkl_ = hilo(chunked(Wk_[:, r * HD:(r + 1) * HD]), WKS)
        wkb_ = np.ascontiguousarray(np.stack([wkh_, wkl_], axis=2))
        wvh_, wvl_ = hilo(chunked(Wv_[:, r * HD:(r + 1) * HD]), WVS)
        wvb_ = np.ascontiguousarray(np.stack([wvh_, wvl_], axis=2))
        wo_r = np.ascontiguousarray(
            Wo_[r * QW:(r + 1) * QW, :].reshape(HPC, 128, D)
            .transpose(1, 0, 2))
        woh_, wol_ = hilo(wo_r, WOS)
        wob_ = np.ascontiguousarray(np.stack([woh_, wol_], axis=1))
        in_maps.append({
            "xb": xb_, "wqb": wqb_, "wkb": wkb_, "wvb": wvb_, "wob": wob_,
            "cosT": cosT, "sinT": sinT, "rmat": rmat, "masks": masks,
        })
    return in_maps


def get_nc():
    if "nc" not in _CACHE:
        _CACHE["nc"] = _build()
    return _CACHE["nc"]


def kernel(x, mask, cos, sin, Wq, Wk, Wv, Wo):
    nc = get_nc()
    in_maps = _prep_inputs(x, cos, sin, Wq, Wk, Wv, Wo)
    res = run_bass_kernel_spmd(nc, in_maps, core_ids=list(range(N_CORES)))
    acc = np.zeros((S, D), np.float32)
    for r in range(N_CORES):
        acc += res.results[r]["out"].astype(np.float32)
    return acc[None]


if __name__ == "__main__":
    print("built:", get_nc() is not None)



# revision 74
# speedup vs baseline: 1.0034x; 1.0034x over previous
"""Grouped-query attention, tensor-parallel over heads across 8 TRN2 NeuronCores.

Problem (hardcoded): x[1,1024,4096] @ Wq/Wk/Wv -> RoPE -> causal GQA
(32 q heads, 8 kv groups, head_dim 128) -> out proj Wo -> [1,1024,4096].

Sharding: core r owns q heads 4r..4r+3 and kv group r (Wq/Wk/Wv column
shards, Wo row shard). Each core computes a full [1024,4096] partial of
the output projection; the host sums the 8 partials (the "all-reduce").

Device kernel (per core): the big GEMMs (Q/K/V projections, out-proj)
run in fp8e4 DoubleRow mode, which processes two 128-deep contraction
chunks per instruction at half the cycles/row of bf16.  Precision is
recovered with a 3-term hi/lo split quantization (x_hi@W_hi + x_lo@W_hi
+ x_hi@W_lo), where hi and lo shares one power-of-2 scale so all terms
accumulate in a single PSUM chain; measured end-to-end error matches
bf16.  The attention core (scores, exp, ctx) stays bf16 with 256-wide
s-blocks and causal tile skipping.  The softmax denominator is computed
off the tensor engine entirely: a DVE tree-sum of the P tiles, a Pool
partition_all_reduce, and a bf16 reciprocal.  The first s-block's
attention stages are pre-emitted into the second projection half's
instruction stream (scores matmuls fill DMA-paced slack; their exps run
on the then-idle Act engine), and output-projection tiles stream out
through partial-region DMAs so only a 512-column transfer trails the
final matmul.
"""

import numpy as np
import ml_dtypes

import concourse.bass as bass
import concourse.bacc as bacc
import concourse.mybir as mybir
import concourse.tile as tile
import concourse.bass_isa as bass_isa
from concourse.bass_utils import run_bass_kernel_spmd

S = 1024          # sequence length
D = 4096          # model dim
H = 32            # query heads (global)
G = 8             # kv groups (global)
HD = 128          # head dim
N_CORES = 8
HPC = H // N_CORES   # 4 query heads per core
QW = HPC * HD        # 512 q-proj cols per core
NDC = D // 128       # 32 contraction chunks
NP = NDC // 2        # 16 DoubleRow chunk pairs
BF = mybir.dt.bfloat16
F8 = mybir.dt.float8e4
F32 = mybir.dt.float32
DR = mybir.MatmulPerfMode.DoubleRow

# quantization scales (powers of 2; hi and lo share the scale so every
# 3-term matmul accumulates in one PSUM chain)
XS = 16.0
WQS = 8192.0        # applied to Wq/sqrt(HD)
WKS = 1024.0
WVS = 1024.0
WOS = 1024.0
CTXS = 16.0
EXP_SHIFT = -6.0    # exp(s - 6): keeps bf16 P comfortably in range
N_WARM = 38         # startup PE warm-up matmuls bridging the DMA fill

_CACHE = {}


def _build():
    nc = bacc.Bacc("TRN2", target_bir_lowering=False, debug=False,
                   num_devices=N_CORES)

    xb = nc.dram_tensor("xb", [128, NDC, 2, S], F8, kind="ExternalInput")
    wqb = nc.dram_tensor("wqb", [128, NDC, 2, QW], F8, kind="ExternalInput")
    wkb = nc.dram_tensor("wkb", [128, NDC, 2, HD], F8, kind="ExternalInput")
    wvb = nc.dram_tensor("wvb", [128, NDC, 2, HD], F8, kind="ExternalInput")
    wob = nc.dram_tensor("wob", [128, 2, HPC, D], F8, kind="ExternalInput")
    cosT = nc.dram_tensor("cosT", [HD, S], BF, kind="ExternalInput")
    sinT = nc.dram_tensor("sinT", [HD, S], BF, kind="ExternalInput")
    rmat = nc.dram_tensor("rmat", [HD, HD], BF, kind="ExternalInput")
    masks = nc.dram_tensor("masks", [128, 512], BF, kind="ExternalInput")
    out = nc.dram_tensor("out", [S, D], BF, kind="ExternalOutput")

    with tile.TileContext(nc) as tc:
        _emit(tc, nc, xb, wqb, wkb, wvb, wob,
              cosT, sinT, rmat, masks, out)
    nc.compile()
    return nc


def _emit(tc, nc, xb, wqb, wkb, wvb, wob,
          cosT, sinT, rmat, masks, out):
    import contextlib
    ctx = contextlib.ExitStack()
    with ctx:
        const = ctx.enter_context(tc.tile_pool(name="const", bufs=1))
        work = ctx.enter_context(tc.tile_pool(name="work", bufs=1))
        tmp = ctx.enter_context(tc.tile_pool(name="tmp", bufs=4))
        pt_pool = ctx.enter_context(tc.tile_pool(name="pt", bufs=8))
        outp = ctx.enter_context(tc.tile_pool(name="outp", bufs=2))
        ps = ctx.enter_context(tc.tile_pool(name="ps", bufs=8, space="PSUM"))

        # ---- DMA emission, ordered to pace the chunk-major PE stream ----
        rmat_sb = const.tile([HD, HD], BF, tag="rmat")
        # rmat memset first: it gates the PE warm-up start
        nc.vector.memset(rmat_sb[:], 0.5)
        ebias = const.tile([128, 1], F32, tag="ebias")
        nc.gpsimd.memset(ebias[:], EXP_SHIFT)

        wk_sb = const.tile([128, NDC, 2, HD], F8, tag="wk")   # dim2: hi/lo

        # PE warm-up bridge: junk matmuls over the startup DMA fill keep the
        # tensor engine's clock-ramp streak alive until real operands land.
        wps = ps.tile([128, 512], F32, tag="ps", name="wps")
        for wi in range(N_WARM):
            nc.tensor.matmul(wps[:, 0:HD], rmat_sb[:], rmat_sb[:],
                             start=(wi == 0), stop=(wi == N_WARM - 1))
        nc.vector.tensor_copy(rmat_sb[:], wps[:, 0:HD])
        nc.sync.dma_start(out=rmat_sb[:], in_=rmat.ap())

        gx4, gq4 = {}, {}
        for c in range(0, NDC, 4):
            gx4[c] = const.tile([128, 4, 2, S], F8, tag=f"x{c//4}", name=f"x{c//4}")
        gxh = {c: gx4[c - c % 4][:, c % 4:c % 4 + 2, 0, :] for c in range(0, NDC, 2)}
        gxl = {c: gx4[c - c % 4][:, c % 4:c % 4 + 2, 1, :] for c in range(0, NDC, 2)}
        gqh, gql = {}, {}
        # half-0 of x, wq, and wk interleaved in consumption order
        for c in range(0, NDC, 4):
            nc.sync.dma_start(out=wk_sb[:, c:c + 4, :, :],
                              in_=wkb.ap()[:, c:c + 4, :, :])
            g = const.tile([128, 4, 2, QW], F8, tag=f"q{c//4}", name=f"q{c//4}")
            gqh[c], gqh[c + 2] = g[:, 0:2, 0, :], g[:, 2:4, 0, :]
            gql[c], gql[c + 2] = g[:, 0:2, 1, :], g[:, 2:4, 1, :]
            if c == 0:
                # fine-grained early groups so the PE stream starts early
                for c2 in (0, 2):
                    nc.sync.dma_start(out=g[:, c2:c2 + 2, :, :],
                                      in_=wqb.ap()[:, c + c2:c + c2 + 2, :, :])
                    nc.sync.dma_start(
                        out=gx4[c][:, c2:c2 + 2, :, 0:512],
                        in_=xb.ap()[:, c + c2:c + c2 + 2, :, 0:512])
            else:
                for c2 in (0, 2):
                    nc.sync.dma_start(out=g[:, c2:c2 + 2, :, :],
                                      in_=wqb.ap()[:, c + c2:c + c2 + 2, :, :])
                    nc.sync.dma_start(
                        out=gx4[c][:, c2:c2 + 2, :, 0:512],
                        in_=xb.ap()[:, c + c2:c + c2 + 2, :, 0:512])
        cos_sb = const.tile([HD, S], BF, tag="cos")
        nc.sync.dma_start(out=cos_sb[:], in_=cosT.ap())
        sin_sb = const.tile([HD, S], BF, tag="sin")
        nc.sync.dma_start(out=sin_sb[:], in_=sinT.ap())
        # half-1 of x
        for c in range(0, NDC, 4):
            nc.sync.dma_start(out=gx4[c][:, :, :, 512:S],
                              in_=xb.ap()[:, c:c + 4, :, 512:S])
        wv_sb = const.tile([128, NDC, 2, HD], F8, tag="wv")
        nc.sync.dma_start(out=wv_sb[:], in_=wvb.ap())
        mask_sb = const.tile([128, 512], BF, tag="mask")
        nc.sync.dma_start(out=mask_sb[:], in_=masks.ap())
        wo_sb = const.tile([128, 2, HPC, D], F8, tag="wo")    # dim1: hi/lo
        for n in range(2):
            sl = slice(n * 2048, (n + 1) * 2048)
            nc.sync.dma_start(out=wo_sb[:, :, :, sl], in_=wob.ap()[:, :, :, sl])

        # persistent activations
        khat = work.tile([HD, S], BF, tag="khat")
        qhat = [work.tile([HD, S], BF, tag=f"qhat{h}", name=f"qhat{h}")
                for h in range(HPC)]
        v_sb = [work.tile([128, HD], BF, tag=f"v{i}", name=f"v{i}")
                for i in range(8)]
        ctx_hi = [work.tile([128, 2, S], F8, tag=f"cth{u}", name=f"cth{u}")
                  for u in range(2)]
        ctx_lo = [work.tile([128, 2, S], F8, tag=f"ctl{u}", name=f"ctl{u}")
                  for u in range(2)]

        # ---- K+Q projections: chunk-major across 5 chains per s-quarter ----
        # Per chunk pair, all five tensors advance their 3-term DoubleRow
        # chains, so the PE stream follows the x/wq DMA arrival order.
        # RoPE for each finished s-half is queued and its PE/Act/DVE work is
        # injected into later quarters' streams (and the v-projection).
        TENS = [("k", khat, lambda c: wk_sb[:, c:c + 2, 0, :],
                 lambda c: wk_sb[:, c:c + 2, 1, :], 1.0 / (XS * WKS))]
        for h in range(HPC):
            hsl = slice(h * HD, (h + 1) * HD)
            TENS.append((f"q{h}", qhat[h],
                         lambda c, s=hsl: gqh[c][:, :, s],
                         lambda c, s=hsl: gql[c][:, :, s], 1.0 / (XS * WQS)))
        raws = {ti: work.tile([HD, S], BF, tag=f"raw{ti}", name=f"raw{ti}")
                for ti in range(5)}

        pend = []   # queued injection closures (ropes, early scores)

        def inject_rope():
            if pend:
                pend.pop(0)()

        def rope_half(ti, half):
            name, dst, _, _, _ = TENS[ti]
            sl = slice(half * 512, (half + 1) * 512)
            t1 = tmp.tile([HD, 512], BF, tag="rope_t1", name="rope_t1", bufs=1)
            nc.vector.tensor_mul(t1[:], raws[ti][:, sl], cos_sb[:, sl])
            rq = ps.tile([HD, 512], F32, tag="ps", name="rq")
            nc.tensor.matmul(rq[:], rmat_sb[:], raws[ti][:, sl],
                             start=True, stop=True)
            t2 = tmp.tile([HD, 512], BF, tag="rope_t2", name="rope_t2", bufs=1)
            nc.vector.tensor_mul(t2[:], rq[:], sin_sb[:, sl])
            nc.vector.tensor_add(dst[:, sl], t1[:], t2[:])

        def v_terms(vpsum, i, p):
            c = 2 * p
            tsl = slice(i * 128, (i + 1) * 128)
            nc.tensor.matmul(vpsum[:], gxh[c][:, :, tsl], wv_sb[:, c:c + 2, 0, :],
                             start=(p == 0), stop=False,
                             perf_mode=DR, skip_group_check=True)
            nc.tensor.matmul(vpsum[:], gxh[c][:, :, tsl], wv_sb[:, c:c + 2, 1, :],
                             start=False, stop=False,
                             perf_mode=DR, skip_group_check=True)
            nc.tensor.matmul(vpsum[:], gxl[c][:, :, tsl], wv_sb[:, c:c + 2, 0, :],
                             start=False, stop=(p == NP - 1),
                             perf_mode=DR, skip_group_check=True)

        pre_pts = {}   # stages pre-emitted into the QK stream
        def emit_scores(h, b):
            ssl = slice(b * 256, (b + 1) * 256)
            pts = []
            for tp in range(b + 1):
                st = ps.tile([128, 512], F32, tag="ps", name="st")
                for i in range(2):
                    t0 = (2 * tp + i) * 128
                    nc.tensor.matmul(st[:, i * 256:(i + 1) * 256],
                                     khat[:, t0:t0 + 128], qhat[h][:, ssl],
                                     start=True, stop=True)
                pt = pt_pool.tile([128, 512], BF, tag="pt", name="pt")
                nc.scalar.activation(pt[:], st[:],
                                     mybir.ActivationFunctionType.Exp,
                                     bias=ebias[:])
                if tp == b:
                    nc.vector.tensor_mul(pt[:], pt[:], mask_sb[:])
                pts.append(pt)
            # Denominator off the PE: DVE tree-sum of P tiles, Pool
            # partition-all-reduce, DVE half-merge + reciprocal. All emitted
            # here so the result has a full pipeline stage of slack.
            if b == 0:
                acc = pts[0]
            else:
                acc = tmp.tile([128, 512], BF, tag="dacc", name="dacc", bufs=2)
                if b == 1:
                    nc.vector.tensor_add(acc[:], pts[0][:], pts[1][:])
                elif b == 2:
                    t01 = tmp.tile([128, 512], BF, tag="dacc2", name="dacc2",
                                   bufs=1)
                    nc.vector.tensor_add(t01[:], pts[0][:], pts[1][:])
                    nc.vector.tensor_add(acc[:], t01[:], pts[2][:])
                else:
                    t01 = tmp.tile([128, 512], BF, tag="dacc2", name="dacc2",
                                   bufs=1)
                    t23 = tmp.tile([128, 512], BF, tag="dacc3", name="dacc3",
                                   bufs=1)
                    nc.vector.tensor_add(t01[:], pts[0][:], pts[1][:])
                    nc.vector.tensor_add(t23[:], pts[2][:], pts[3][:])
                    nc.vector.tensor_add(acc[:], t01[:], t23[:])
            denb = tmp.tile([128, 512], BF, tag="denb", name="denb", bufs=2)
            nc.gpsimd.partition_all_reduce(denb[:], acc[:], 128,
                                           bass_isa.ReduceOp.add)
            rec = tmp.tile([128, 256], BF, tag="rec", name="rec", bufs=5)
            with nc.allow_low_precision(reason="den scale, 0.2% rel is fine"):
                nc.vector.tensor_add(rec[:], denb[:, 0:256],
                                     denb[:, 256:512])
                nc.vector.reciprocal(rec[:], rec[:])
            return pts, rec


        v_first = []   # first v-chains, interleaved with half-1 descales

        for half in (0, 1):
            hoff = half * 512
            chains = [ps.tile([128, 512], F32, tag="ps", name=f"ch{ti}")
                      for ti in range(5)]
            for p in range(NP):
                c = 2 * p
                for term in range(3):
                    for ti, (_, _, whi, wlo, _) in enumerate(TENS):
                        pp = chains[ti]
                        w = whi(c) if term != 1 else wlo(c)
                        for q in range(2):
                            ssl = slice(hoff + q * 256, hoff + (q + 1) * 256)
                            osl = slice(q * 256, (q + 1) * 256)
                            xop = gxl[c] if term == 2 else gxh[c]
                            nc.tensor.matmul(
                                pp[:, osl], w, xop[:, :, ssl],
                                start=(p == 0 and term == 0 and q == 0),
                                stop=(p == NP - 1 and term == 2 and q == 1),
                                perf_mode=DR, skip_group_check=True)
                if p in (2, 4, 6, 8, 10, 12, 14):
                    inject_rope()
            if half == 0:
                for ti, (_, _, _, _, descale) in enumerate(TENS):
                    nc.scalar.activation(raws[ti][:, 0:512], chains[ti][:],
                                         mybir.ActivationFunctionType.Copy,
                                         scale=descale)
                def early_scores(h):
                    pre_pts[(h, 0)] = emit_scores(h, 0)
                pend.extend([
                    lambda: rope_half(0, 0),    # k
                    lambda: rope_half(1, 0),    # q0
                    lambda: rope_half(2, 0),    # q1
                    lambda: early_scores(0),
                    lambda: rope_half(3, 0),    # q2
                    lambda: early_scores(1),
                    lambda: rope_half(4, 0),    # q3
                ])
            else:
                # defer the 5 serial ACT descales until after the first
                # attention stage's exps are queued (ACT priority)
                def half1_tail(chains=chains):
                    for ti, (_, _, _, _, descale) in enumerate(TENS):
                        nc.scalar.activation(
                            raws[ti][:, 512:S], chains[ti][:],
                            mybir.ActivationFunctionType.Copy, scale=descale)
                        pend.append(lambda t=ti: rope_half(t, 1))
                v_first.append(half1_tail)

        # ---- V projection: emitted as PE filler inside early attention ----
        def v_chain(i):
            tsl = slice(i * 128, (i + 1) * 128)
            vp = ps.tile([128, HD], F32, tag="ps", name="vp")
            for p in range(NP):
                c = 2 * p
                nc.tensor.matmul(vp[:], gxh[c][:, :, tsl], wv_sb[:, c:c + 2, 0, :],
                                 start=(p == 0), stop=False, perf_mode=DR)
            for p in range(NP):
                c = 2 * p
                nc.tensor.matmul(vp[:], gxh[c][:, :, tsl], wv_sb[:, c:c + 2, 1, :],
                                 start=False, stop=False, perf_mode=DR)
            for p in range(NP):
                c = 2 * p
                nc.tensor.matmul(vp[:], gxl[c][:, :, tsl], wv_sb[:, c:c + 2, 0, :],
                                 start=False, stop=(p == NP - 1), perf_mode=DR)
            nc.scalar.activation(v_sb[i][:], vp[:],
                                 mybir.ActivationFunctionType.Copy,
                                 scale=1.0 / (XS * WVS))
            inject_rope()

        pre_pts[(2, 0)] = emit_scores(2, 0)
        v_first[0]()          # the deferred half-1 descales
        v_chain(0)
        pre_pts[(3, 0)] = emit_scores(3, 0)
        v_chain(1)
        vq = list(range(2, 8))

        def emit_denctx(h, b, pts, rec):
            ssl = slice(b * 256, (b + 1) * 256)
            cx = ps.tile([HD, 256], F32, tag="ps", name="cx")
            n_mm = 2 * (b + 1)
            k = 0
            for tp, pt in enumerate(pts):
                for i in range(2):
                    if tp == b and i == 1:
                        # diagonal pair: tokens 128-255 only reach s>=128
                        nc.tensor.matmul(cx[:, 128:256], v_sb[2 * tp + i][:],
                                         pt[:, 384:512],
                                         start=False, stop=(k == n_mm - 1))
                    else:
                        nc.tensor.matmul(cx[:], v_sb[2 * tp + i][:],
                                         pt[:, i * 256:(i + 1) * 256],
                                         start=(k == 0), stop=(k == n_mm - 1))
                    k += 1
            ctxn = tmp.tile([HD, 256], F32, tag="ctxn", name="ctxn", bufs=2)
            nc.vector.scalar_tensor_tensor(
                ctxn[:], cx[:], CTXS, rec[:],
                op0=mybir.AluOpType.mult, op1=mybir.AluOpType.mult)
            u, par = divmod(h, 2)
            nc.vector.tensor_copy(ctx_hi[u][:, par, ssl], ctxn[:])
            nc.vector.tensor_sub(ctx_lo[u][:, par, ssl], ctxn[:],
                                 ctx_hi[u][:, par, ssl])

        descale = 1.0 / (CTXS * WOS)

        def emit_outproj(t8, last=False):
            tsl = slice(t8 * 128, (t8 + 1) * 128)
            for n2 in range(2):
                ot = outp.tile([128, 2048], BF, tag="ot", name="ot")
                # partial-region DMAs [1024|512|512]; the very last tile
                # uses [1536|512] so only one HWDGE generation trails the
                # final matmul
                if last and n2 == 1:
                    dma_after = {1: (0, 1024), 3: (1024, 1024)}
                else:
                    dma_after = {1: (0, 1024), 2: (1024, 512),
                                 3: (1536, 512)}
                for half2 in range(4):
                    op = ps.tile([128, 512], F32, tag="ps", name="op")
                    for sub2 in range(2):
                        n = 8 * n2 + 2 * half2 + sub2
                        nsl = slice(n * 256, (n + 1) * 256)
                        osl = slice(sub2 * 256, (sub2 + 1) * 256)
                        k = 0
                        for u in range(2):
                            for chi, whi in ((ctx_hi, 0), (ctx_lo, 0),
                                             (ctx_hi, 1)):
                                nc.tensor.matmul(
                                    op[:, osl], chi[u][:, :, tsl],
                                    wo_sb[:, whi, 2 * u:2 * u + 2, nsl],
                                    start=(k == 0 and sub2 == 0),
                                    stop=(k == 5 and sub2 == 1),
                                    perf_mode=DR, skip_group_check=True)
                                k += 1
                    osl2 = slice(half2 * 512, (half2 + 1) * 512)
                    if half2 % 2 == 0:
                        nc.vector.tensor_scalar_mul(ot[:, osl2], op[:],
                                                    descale)
                    else:
                        nc.scalar.activation(
                            ot[:, osl2], op[:],
                            mybir.ActivationFunctionType.Copy,
                            scale=descale)
                    if half2 in dma_after:
                        o0, ow = dma_after[half2]
                        nc.sync.dma_start(
                            out=out.ap()[tsl, n2 * 2048 + o0:
                                         n2 * 2048 + o0 + ow],
                            in_=ot[:, o0:o0 + ow])

        stages = [(h, b) for b in range(4) for h in range(HPC)]
        prev = None
        outq = []
        cooldown = 0
        for si, hb in enumerate(stages):
            if hb in pre_pts:
                pts, dacc = pre_pts.pop(hb)
            else:
                pts, dacc = emit_scores(*hb)
            todo = None
            if cooldown:
                cooldown -= 1
            elif outq:
                todo = outq.pop(0)
            if prev is not None:
                (ph, pb), ppts, pdacc = prev
                emit_denctx(ph, pb, ppts, pdacc)
                if ph == HPC - 1:
                    outq.extend([2 * pb + 1, 2 * pb])
                    cooldown = 1
            if todo is not None:
                emit_outproj(todo)
            inject_rope()
            for _ in range(2 if si < 3 else 1):
                if vq:
                    v_chain(vq.pop(0))
            if si + 1 < len(stages):
                nxt = stages[si + 1]
                if nxt not in pre_pts:
                    pre_pts[nxt] = emit_scores(*nxt)
            prev = (hb, pts, dacc)
        (ph, pb), ppts, pdacc = prev
        emit_denctx(ph, pb, ppts, pdacc)
        outq.extend([2 * pb + 1, 2 * pb])
        for qi, t8 in enumerate(outq):
            emit_outproj(t8, last=(qi == len(outq) - 1))


def _prep_inputs(x, cos, sin, Wq, Wk, Wv, Wo):
    """Host-side shard + hi/lo fp8 quantization. Returns per-core inputs."""
    bf = ml_dtypes.bfloat16
    f8 = ml_dtypes.float8_e4m3

    def hilo(a, s):
        hi = np.asarray(a * s, np.float32).astype(f8)
        lo = (np.asarray(a * s, np.float32) - hi.astype(np.float32)).astype(f8)
        return hi, lo

    x2 = np.asarray(x, np.float32).reshape(S, D)
    xTh = np.ascontiguousarray(x2.T).reshape(NDC, 128, S).transpose(1, 0, 2)
    xh_, xl_ = hilo(np.ascontiguousarray(xTh), XS)
    xb_ = np.ascontiguousarray(np.stack([xh_, xl_], axis=2))  # [128,NDC,2,S]

    cosT = np.ascontiguousarray(np.asarray(cos, np.float32).T).astype(bf)
    sinT = np.ascontiguousarray(np.asarray(sin, np.float32).T).astype(bf)

    rmat = np.zeros((HD, HD), np.float32)
    half = HD // 2
    rmat[np.arange(half), np.arange(half) + half] = 1.0
    rmat[np.arange(half) + half, np.arange(half)] = -1.0
    rmat = rmat.astype(bf)

    # diagonal pair mask: keep when t_local (= i*128 + p) <= s_local
    lt = np.arange(128)[:, None]
    ls = np.arange(256)[None, :]
    masks = np.concatenate([(lt + 128 * i <= ls) for i in range(2)], axis=1)
    masks = np.ascontiguousarray(masks).astype(bf)     # [128, 512]

    scale = 1.0 / np.sqrt(np.float32(HD))
    Wq_ = np.asarray(Wq, np.float32) * scale
    Wk_ = np.asarray(Wk, np.float32)
    Wv_ = np.asarray(Wv, np.float32)
    Wo_ = np.asarray(Wo, np.float32)

    def chunked(w):  # [D, m] -> [128, NDC, m]
        m = w.shape[1]
        return np.ascontiguousarray(
            w.reshape(NDC, 128, m).transpose(1, 0, 2))

    in_maps = []
    for r in range(N_CORES):
        wqh_, wql_ = hilo(chunked(Wq_[:, r * QW:(r + 1) * QW]), WQS)
        wqb_ = np.ascontiguousarray(np.stack([wqh_, wql_], axis=2))
        wkh_, wkl_ = hilo(chunked(Wk_[:, r * HD:(r + 1) * HD]), WKS)
        wkb_ = np.ascontiguousarray(np.stack([wkh_, wkl_], axis=2))
        wvh_, wvl_ = hilo(chunked(Wv_[:, r * HD:(r + 1) * HD]), WVS)
        wvb_ = np.ascontiguousarray(np.stack([wvh_, wvl_], axis=2))
        wo_r = np.ascontiguousarray(
            Wo_[r * QW:(r + 1) * QW, :].reshape(HPC, 128, D)
            .transpose(1, 0, 2))
        woh_, wol_ = hilo(wo_r, WOS)
        wob_ = np.ascontiguousarray(np.stack([woh_, wol_], axis=1))
        in_maps.append({
            "xb": xb_, "wqb": wqb_, "wkb": wkb_, "wvb": wvb_, "wob": wob_,
            "cosT": cosT, "sinT": sinT, "rmat": rmat, "masks": masks,
        })
    return in_maps


def get_nc():
    if "nc" not in _CACHE:
        _CACHE["nc"] = _build()
    return _CACHE["nc"]


def kernel(x, mask, cos, sin, Wq, Wk, Wv, Wo):
    nc = get_nc()
    in_maps = _prep_inputs(x, cos, sin, Wq, Wk, Wv, Wo)
    res = run_bass_kernel_spmd(nc, in_maps, core_ids=list(range(N_CORES)))
    acc = np.zeros((S, D), np.float32)
    for r in range(N_CORES):
        acc += res.results[r]["out"].astype(np.float32)
    return acc[None]


if __name__ == "__main__":
    print("built:", get_nc() is not None)



# revision 81
# speedup vs baseline: 1.0079x; 1.0045x over previous
"""Grouped-query attention, tensor-parallel over heads across 8 TRN2 NeuronCores.

Problem (hardcoded): x[1,1024,4096] @ Wq/Wk/Wv -> RoPE -> causal GQA
(32 q heads, 8 kv groups, head_dim 128) -> out proj Wo -> [1,1024,4096].

Sharding: core r owns q heads 4r..4r+3 and kv group r (Wq/Wk/Wv column
shards, Wo row shard). Each core computes a full [1024,4096] partial of
the output projection; the host sums the 8 partials (the "all-reduce").

Device kernel (per core): the big GEMMs (Q/K/V projections, out-proj)
run in fp8e4 DoubleRow mode, which processes two 128-deep contraction
chunks per instruction at half the cycles/row of bf16.  Precision is
recovered with a 3-term hi/lo split quantization (x_hi@W_hi + x_lo@W_hi
+ x_hi@W_lo), where hi and lo shares one power-of-2 scale so all terms
accumulate in a single PSUM chain; measured end-to-end error matches
bf16.  The attention core (scores, exp, ctx) stays bf16 with 256-wide
s-blocks and causal tile skipping.  The softmax denominator is computed
off the tensor engine entirely: a DVE tree-sum of the P tiles, a Pool
partition_all_reduce, and a bf16 reciprocal.  The first s-block's
attention stages are pre-emitted into the second projection half's
instruction stream (scores matmuls fill DMA-paced slack; their exps run
on the then-idle Act engine), and output-projection tiles stream out
through partial-region DMAs so only a 512-column transfer trails the
final matmul.
"""

import numpy as np
import ml_dtypes

import concourse.bass as bass
import concourse.bacc as bacc
import concourse.mybir as mybir
import concourse.tile as tile
import concourse.bass_isa as bass_isa
from concourse.bass_utils import run_bass_kernel_spmd

S = 1024          # sequence length
D = 4096          # model dim
H = 32            # query heads (global)
G = 8             # kv groups (global)
HD = 128          # head dim
N_CORES = 8
HPC = H // N_CORES   # 4 query heads per core
QW = HPC * HD        # 512 q-proj cols per core
NDC = D // 128       # 32 contraction chunks
NP = NDC // 2        # 16 DoubleRow chunk pairs
BF = mybir.dt.bfloat16
F8 = mybir.dt.float8e4
F32 = mybir.dt.float32
DR = mybir.MatmulPerfMode.DoubleRow

# quantization scales (powers of 2; hi and lo share the scale so every
# 3-term matmul accumulates in one PSUM chain)
XS = 16.0
WQS = 8192.0        # applied to Wq/sqrt(HD)
WKS = 1024.0
WVS = 1024.0
WOS = 1024.0
CTXS = 16.0
EXP_SHIFT = -6.0    # exp(s - 6): keeps bf16 P comfortably in range
N_WARM = 38         # startup PE warm-up matmuls bridging the DMA fill

_CACHE = {}


def _build():
    nc = bacc.Bacc("TRN2", target_bir_lowering=False, debug=False,
                   num_devices=N_CORES)

    xb = nc.dram_tensor("xb", [128, NDC, 2, S], F8, kind="ExternalInput")
    wqb = nc.dram_tensor("wqb", [128, NDC, 2, QW], F8, kind="ExternalInput")
    wkb = nc.dram_tensor("wkb", [128, NDC, 2, HD], F8, kind="ExternalInput")
    wvb = nc.dram_tensor("wvb", [128, NDC, 2, HD], F8, kind="ExternalInput")
    wob = nc.dram_tensor("wob", [128, 2, HPC, D], F8, kind="ExternalInput")
    cosT = nc.dram_tensor("cosT", [HD, S], BF, kind="ExternalInput")
    sinT = nc.dram_tensor("sinT", [HD, S], BF, kind="ExternalInput")
    rmat = nc.dram_tensor("rmat", [HD, HD], BF, kind="ExternalInput")
    masks = nc.dram_tensor("masks", [128, 512], BF, kind="ExternalInput")
    out = nc.dram_tensor("out", [S, D], BF, kind="ExternalOutput")

    with tile.TileContext(nc) as tc:
        _emit(tc, nc, xb, wqb, wkb, wvb, wob,
              cosT, sinT, rmat, masks, out)
    nc.compile()
    return nc


def _emit(tc, nc, xb, wqb, wkb, wvb, wob,
          cosT, sinT, rmat, masks, out):
    import contextlib
    ctx = contextlib.ExitStack()
    with ctx:
        const = ctx.enter_context(tc.tile_pool(name="const", bufs=1))
        work = ctx.enter_context(tc.tile_pool(name="work", bufs=1))
        tmp = ctx.enter_context(tc.tile_pool(name="tmp", bufs=4))
        pt_pool = ctx.enter_context(tc.tile_pool(name="pt", bufs=8))
        outp = ctx.enter_context(tc.tile_pool(name="outp", bufs=2))
        ps = ctx.enter_context(tc.tile_pool(name="ps", bufs=8, space="PSUM"))

        # ---- DMA emission, ordered to pace the chunk-major PE stream ----
        rmat_sb = const.tile([HD, HD], BF, tag="rmat")
        # rmat memset first: it gates the PE warm-up start
        nc.vector.memset(rmat_sb[:], 0.5)
        ebias = const.tile([128, 1], F32, tag="ebias")
        nc.gpsimd.memset(ebias[:], EXP_SHIFT)

        wk_sb = const.tile([128, NDC, 2, HD], F8, tag="wk")   # dim2: hi/lo

        # PE warm-up bridge: junk matmuls over the startup DMA fill keep the
        # tensor engine's clock-ramp streak alive until real operands land.
        wps = ps.tile([128, 512], F32, tag="ps", name="wps")
        for wi in range(N_WARM):
            nc.tensor.matmul(wps[:, 0:HD], rmat_sb[:], rmat_sb[:],
                             start=(wi == 0), stop=(wi == N_WARM - 1))
        nc.vector.tensor_copy(rmat_sb[:], wps[:, 0:HD])
        nc.sync.dma_start(out=rmat_sb[:], in_=rmat.ap())

        gx4, gq4 = {}, {}
        for c in range(0, NDC, 4):
            gx4[c] = const.tile([128, 4, 2, S], F8, tag=f"x{c//4}", name=f"x{c//4}")
        gxh = {c: gx4[c - c % 4][:, c % 4:c % 4 + 2, 0, :] for c in range(0, NDC, 2)}
        gxl = {c: gx4[c - c % 4][:, c % 4:c % 4 + 2, 1, :] for c in range(0, NDC, 2)}
        gqh, gql = {}, {}
        # half-0 of x, wq, and wk interleaved in consumption order
        for c in range(0, NDC, 4):
            nc.sync.dma_start(out=wk_sb[:, c:c + 4, :, :],
                              in_=wkb.ap()[:, c:c + 4, :, :])
            g = const.tile([128, 4, 2, QW], F8, tag=f"q{c//4}", name=f"q{c//4}")
            gqh[c], gqh[c + 2] = g[:, 0:2, 0, :], g[:, 2:4, 0, :]
            gql[c], gql[c + 2] = g[:, 0:2, 1, :], g[:, 2:4, 1, :]
            if c == 0:
                # fine-grained early groups so the PE stream starts early
                for c2 in (0, 2):
                    nc.sync.dma_start(out=g[:, c2:c2 + 2, :, :],
                                      in_=wqb.ap()[:, c + c2:c + c2 + 2, :, :])
                    nc.sync.dma_start(
                        out=gx4[c][:, c2:c2 + 2, :, 0:512],
                        in_=xb.ap()[:, c + c2:c + c2 + 2, :, 0:512])
            else:
                for c2 in (0, 2):
                    nc.sync.dma_start(out=g[:, c2:c2 + 2, :, :],
                                      in_=wqb.ap()[:, c + c2:c + c2 + 2, :, :])
                    nc.sync.dma_start(
                        out=gx4[c][:, c2:c2 + 2, :, 0:512],
                        in_=xb.ap()[:, c + c2:c + c2 + 2, :, 0:512])
        cos_sb = const.tile([HD, S], BF, tag="cos")
        nc.sync.dma_start(out=cos_sb[:], in_=cosT.ap())
        sin_sb = const.tile([HD, S], BF, tag="sin")
        nc.sync.dma_start(out=sin_sb[:], in_=sinT.ap())
        # half-1 of x
        for c in range(0, NDC, 4):
            nc.sync.dma_start(out=gx4[c][:, :, :, 512:S],
                              in_=xb.ap()[:, c:c + 4, :, 512:S])
        wv_sb = const.tile([128, NDC, 2, HD], F8, tag="wv")
        nc.sync.dma_start(out=wv_sb[:], in_=wvb.ap())
        mask_sb = const.tile([128, 512], BF, tag="mask")
        nc.sync.dma_start(out=mask_sb[:], in_=masks.ap())
        wo_sb = const.tile([128, 2, HPC, D], F8, tag="wo")    # dim1: hi/lo
        for n in range(2):
            sl = slice(n * 2048, (n + 1) * 2048)
            nc.sync.dma_start(out=wo_sb[:, :, :, sl], in_=wob.ap()[:, :, :, sl])

        # persistent activations
        khat = work.tile([HD, S], BF, tag="khat")
        qhat = [work.tile([HD, S], BF, tag=f"qhat{h}", name=f"qhat{h}")
                for h in range(HPC)]
        v_sb = [work.tile([128, HD], BF, tag=f"v{i}", name=f"v{i}")
                for i in range(8)]
        ctx_hi = [work.tile([128, 2, S], F8, tag=f"cth{u}", name=f"cth{u}")
                  for u in range(2)]
        ctx_lo = [work.tile([128, 2, S], F8, tag=f"ctl{u}", name=f"ctl{u}")
                  for u in range(2)]

        # ---- K+Q projections: chunk-major across 5 chains per s-quarter ----
        # Per chunk pair, all five tensors advance their 3-term DoubleRow
        # chains, so the PE stream follows the x/wq DMA arrival order.
        # RoPE for each finished s-half is queued and its PE/Act/DVE work is
        # injected into later quarters' streams (and the v-projection).
        TENS = [("k", khat, lambda c: wk_sb[:, c:c + 2, 0, :],
                 lambda c: wk_sb[:, c:c + 2, 1, :], 1.0 / (XS * WKS))]
        for h in range(HPC):
            hsl = slice(h * HD, (h + 1) * HD)
            TENS.append((f"q{h}", qhat[h],
                         lambda c, s=hsl: gqh[c][:, :, s],
                         lambda c, s=hsl: gql[c][:, :, s], 1.0 / (XS * WQS)))
        raws = {ti: work.tile([HD, S], BF, tag=f"raw{ti}", name=f"raw{ti}")
                for ti in range(5)}

        pend = []   # queued injection closures (ropes, early scores)

        def inject_rope():
            if pend:
                pend.pop(0)()

        def rope_half(ti, half):
            name, dst, _, _, _ = TENS[ti]
            sl = slice(half * 512, (half + 1) * 512)
            t1 = tmp.tile([HD, 512], BF, tag="rope_t1", name="rope_t1", bufs=1)
            nc.vector.tensor_mul(t1[:], raws[ti][:, sl], cos_sb[:, sl])
            rq = ps.tile([HD, 512], F32, tag="ps", name="rq")
            nc.tensor.matmul(rq[:], rmat_sb[:], raws[ti][:, sl],
                             start=True, stop=True)
            t2 = tmp.tile([HD, 512], BF, tag="rope_t2", name="rope_t2", bufs=1)
            nc.vector.tensor_mul(t2[:], rq[:], sin_sb[:, sl])
            nc.vector.tensor_add(dst[:, sl], t1[:], t2[:])

        def v_terms(vpsum, i, p):
            c = 2 * p
            tsl = slice(i * 128, (i + 1) * 128)
            nc.tensor.matmul(vpsum[:], gxh[c][:, :, tsl], wv_sb[:, c:c + 2, 0, :],
                             start=(p == 0), stop=False,
                             perf_mode=DR, skip_group_check=True)
            nc.tensor.matmul(vpsum[:], gxh[c][:, :, tsl], wv_sb[:, c:c + 2, 1, :],
                             start=False, stop=False,
                             perf_mode=DR, skip_group_check=True)
            nc.tensor.matmul(vpsum[:], gxl[c][:, :, tsl], wv_sb[:, c:c + 2, 0, :],
                             start=False, stop=(p == NP - 1),
                             perf_mode=DR, skip_group_check=True)

        pre_pts = {}   # stages pre-emitted into the QK stream
        def emit_scores(h, b):
            ssl = slice(b * 256, (b + 1) * 256)
            pts = []
            for tp in range(b + 1):
                st = ps.tile([128, 512], F32, tag="ps", name="st")
                for i in range(2):
                    t0 = (2 * tp + i) * 128
                    nc.tensor.matmul(st[:, i * 256:(i + 1) * 256],
                                     khat[:, t0:t0 + 128], qhat[h][:, ssl],
                                     start=True, stop=True)
                pt = pt_pool.tile([128, 512], BF, tag="pt", name="pt")
                nc.scalar.activation(pt[:], st[:],
                                     mybir.ActivationFunctionType.Exp,
                                     bias=ebias[:])
                if tp == b:
                    nc.vector.tensor_mul(pt[:], pt[:], mask_sb[:])
                pts.append(pt)
            # Denominator off the PE: DVE tree-sum of P tiles, Pool
            # partition-all-reduce, DVE half-merge + reciprocal. All emitted
            # here so the result has a full pipeline stage of slack.
            if b == 0:
                acc = pts[0]
            else:
                acc = tmp.tile([128, 512], BF, tag="dacc", name="dacc", bufs=2)
                if b == 1:
                    nc.vector.tensor_add(acc[:], pts[0][:], pts[1][:])
                elif b == 2:
                    t01 = tmp.tile([128, 512], BF, tag="dacc2", name="dacc2",
                                   bufs=1)
                    nc.vector.tensor_add(t01[:], pts[0][:], pts[1][:])
                    nc.vector.tensor_add(acc[:], t01[:], pts[2][:])
                else:
                    t01 = tmp.tile([128, 512], BF, tag="dacc2", name="dacc2",
                                   bufs=1)
                    t23 = tmp.tile([128, 512], BF, tag="dacc3", name="dacc3",
                                   bufs=1)
                    nc.vector.tensor_add(t01[:], pts[0][:], pts[1][:])
                    nc.vector.tensor_add(t23[:], pts[2][:], pts[3][:])
                    nc.vector.tensor_add(acc[:], t01[:], t23[:])
            denb = tmp.tile([128, 512], BF, tag="denb", name="denb", bufs=2)
            nc.gpsimd.partition_all_reduce(denb[:], acc[:], 128,
                                           bass_isa.ReduceOp.add)
            rec = tmp.tile([128, 256], BF, tag="rec", name="rec", bufs=5)
            with nc.allow_low_precision(reason="den scale, 0.2% rel is fine"):
                nc.vector.tensor_add(rec[:], denb[:, 0:256],
                                     denb[:, 256:512])
                nc.vector.reciprocal(rec[:], rec[:])
            return pts, rec


        v_first = []   # first v-chains, interleaved with half-1 descales

        for half in (0, 1):
            hoff = half * 512
            chains = [ps.tile([128, 512], F32, tag="ps", name=f"ch{ti}")
                      for ti in range(5)]
            for p in range(NP):
                c = 2 * p
                for term in range(3):
                    for ti, (_, _, whi, wlo, _) in enumerate(TENS):
                        pp = chains[ti]
                        w = whi(c) if term != 1 else wlo(c)
                        for q in range(2):
                            ssl = slice(hoff + q * 256, hoff + (q + 1) * 256)
                            osl = slice(q * 256, (q + 1) * 256)
                            xop = gxl[c] if term == 2 else gxh[c]
                            nc.tensor.matmul(
                                pp[:, osl], w, xop[:, :, ssl],
                                start=(p == 0 and term == 0 and q == 0),
                                stop=(p == NP - 1 and term == 2 and q == 1),
                                perf_mode=DR, skip_group_check=True)
                if p in (2, 4, 6, 8, 10, 12, 14):
                    inject_rope()
            if half == 0:
                for ti, (_, _, _, _, descale) in enumerate(TENS):
                    nc.scalar.activation(raws[ti][:, 0:512], chains[ti][:],
                                         mybir.ActivationFunctionType.Copy,
                                         scale=descale)
                def early_scores(h):
                    pre_pts[(h, 0)] = emit_scores(h, 0)
                pend.extend([
                    lambda: rope_half(0, 0),    # k
                    lambda: rope_half(1, 0),    # q0
                    lambda: rope_half(2, 0),    # q1
                    lambda: early_scores(0),
                    lambda: rope_half(3, 0),    # q2
                    lambda: early_scores(1),
                    lambda: rope_half(4, 0),    # q3
                ])
            else:
                # defer the 5 serial ACT descales until after the first
                # attention stage's exps are queued (ACT priority)
                def half1_tail(chains=chains):
                    for ti, (_, _, _, _, descale) in enumerate(TENS):
                        nc.scalar.activation(
                            raws[ti][:, 512:S], chains[ti][:],
                            mybir.ActivationFunctionType.Copy, scale=descale)
                        pend.append(lambda t=ti: rope_half(t, 1))
                v_first.append(half1_tail)

        # ---- V projection: emitted as PE filler inside early attention ----
        def v_chain(i):
            tsl = slice(i * 128, (i + 1) * 128)
            vp = ps.tile([128, HD], F32, tag="ps", name="vp")
            for p in range(NP):
                c = 2 * p
                nc.tensor.matmul(vp[:], gxh[c][:, :, tsl], wv_sb[:, c:c + 2, 0, :],
                                 start=(p == 0), stop=False, perf_mode=DR)
            for p in range(NP):
                c = 2 * p
                nc.tensor.matmul(vp[:], gxh[c][:, :, tsl], wv_sb[:, c:c + 2, 1, :],
                                 start=False, stop=False, perf_mode=DR)
            for p in range(NP):
                c = 2 * p
                nc.tensor.matmul(vp[:], gxl[c][:, :, tsl], wv_sb[:, c:c + 2, 0, :],
                                 start=False, stop=(p == NP - 1), perf_mode=DR)
            nc.scalar.activation(v_sb[i][:], vp[:],
                                 mybir.ActivationFunctionType.Copy,
                                 scale=1.0 / (XS * WVS))
            inject_rope()

        pre_pts[(2, 0)] = emit_scores(2, 0)
        v_first[0]()          # the deferred half-1 descales
        v_chain(0)
        pre_pts[(3, 0)] = emit_scores(3, 0)
        v_chain(1)
        vq = list(range(2, 8))

        def emit_denctx(h, b, pts, rec):
            ssl = slice(b * 256, (b + 1) * 256)
            cx = ps.tile([HD, 256], F32, tag="ps", name="cx")
            n_mm = 2 * (b + 1)
            k = 0
            for tp, pt in enumerate(pts):
                for i in range(2):
                    if tp == b and i == 1:
                        # diagonal pair: tokens 128-255 only reach s>=128
                        nc.tensor.matmul(cx[:, 128:256], v_sb[2 * tp + i][:],
                                         pt[:, 384:512],
                                         start=False, stop=(k == n_mm - 1))
                    else:
                        nc.tensor.matmul(cx[:], v_sb[2 * tp + i][:],
                                         pt[:, i * 256:(i + 1) * 256],
                                         start=(k == 0), stop=(k == n_mm - 1))
                    k += 1
            ctxn = tmp.tile([HD, 256], F32, tag="ctxn", name="ctxn", bufs=2)
            nc.vector.scalar_tensor_tensor(
                ctxn[:], cx[:], CTXS, rec[:],
                op0=mybir.AluOpType.mult, op1=mybir.AluOpType.mult)
            u, par = divmod(h, 2)
            nc.vector.tensor_copy(ctx_hi[u][:, par, ssl], ctxn[:])
            nc.vector.tensor_sub(ctx_lo[u][:, par, ssl], ctxn[:],
                                 ctx_hi[u][:, par, ssl])

        descale = 1.0 / (CTXS * WOS)

        def emit_outproj(t8, last=False):
            tsl = slice(t8 * 128, (t8 + 1) * 128)
            for n2 in range(2):
                ot = outp.tile([128, 2048], BF, tag="ot", name="ot")
                # partial-region DMAs [1024|512|512]; the very last tile
                # uses [1536|512] so only one HWDGE generation trails the
                # final matmul
                if last and n2 == 1:
                    dma_after = {1: (0, 1024), 3: (1024, 1024)}
                else:
                    dma_after = {1: (0, 1024), 2: (1024, 512),
                                 3: (1536, 512)}
                for half2 in range(4):
                    op = ps.tile([128, 512], F32, tag="ps", name="op")
                    for sub2 in range(2):
                        n = 8 * n2 + 2 * half2 + sub2
                        nsl = slice(n * 256, (n + 1) * 256)
                        osl = slice(sub2 * 256, (sub2 + 1) * 256)
                        k = 0
                        for u in range(2):
                            for chi, whi in ((ctx_hi, 0), (ctx_lo, 0),
                                             (ctx_hi, 1)):
                                nc.tensor.matmul(
                                    op[:, osl], chi[u][:, :, tsl],
                                    wo_sb[:, whi, 2 * u:2 * u + 2, nsl],
                                    start=(k == 0 and sub2 == 0),
                                    stop=(k == 5 and sub2 == 1),
                                    perf_mode=DR, skip_group_check=True)
                                k += 1
                    osl2 = slice(half2 * 512, (half2 + 1) * 512)
                    if half2 == 0:
                        nc.vector.tensor_scalar_mul(ot[:, osl2], op[:],
                                                    descale)
                    else:
                        nc.scalar.activation(
                            ot[:, osl2], op[:],
                            mybir.ActivationFunctionType.Copy,
                            scale=descale)
                    if half2 in dma_after:
                        o0, ow = dma_after[half2]
                        nc.sync.dma_start(
                            out=out.ap()[tsl, n2 * 2048 + o0:
                                         n2 * 2048 + o0 + ow],
                            in_=ot[:, o0:o0 + ow])

        stages = [(h, b) for b in range(4) for h in range(HPC)]
        prev = None
        outq = []
        cooldown = 0
        for si, hb in enumerate(stages):
            if hb in pre_pts:
                pts, dacc = pre_pts.pop(hb)
            else:
                pts, dacc = emit_scores(*hb)
            todo = None
            if cooldown:
                cooldown -= 1
            elif outq:
                todo = outq.pop(0)
            if prev is not None:
                (ph, pb), ppts, pdacc = prev
                emit_denctx(ph, pb, ppts, pdacc)
                if ph == HPC - 1:
                    outq.extend([2 * pb + 1, 2 * pb])
                    cooldown = 1
            if todo is not None:
                emit_outproj(todo)
            inject_rope()
            for _ in range(2 if si < 3 else 1):
                if vq:
                    v_chain(vq.pop(0))
            if si + 1 < len(stages):
                nxt = stages[si + 1]
                if nxt not in pre_pts:
                    pre_pts[nxt] = emit_scores(*nxt)
            prev = (hb, pts, dacc)
        (ph, pb), ppts, pdacc = prev
        emit_denctx(ph, pb, ppts, pdacc)
        outq.extend([2 * pb + 1, 2 * pb])
        for qi, t8 in enumerate(outq):
            emit_outproj(t8, last=(qi == len(outq) - 1))


def _prep_inputs(x, cos, sin, Wq, Wk, Wv, Wo):
    """Host-side shard + hi/lo fp8 quantization. Returns per-core inputs."""
    bf = ml_dtypes.bfloat16
    f8 = ml_dtypes.float8_e4m3

    def hilo(a, s):
        hi = np.asarray(a * s, np.float32).astype(f8)
        lo = (np.asarray(a * s, np.float32) - hi.astype(np.float32)).astype(f8)
        return hi, lo

    x2 = np.asarray(x, np.float32).reshape(S, D)
    xTh = np.ascontiguousarray(x2.T).reshape(NDC, 128, S).transpose(1, 0, 2)
    xh_, xl_ = hilo(np.ascontiguousarray(xTh), XS)
    xb_ = np.ascontiguousarray(np.stack([xh_, xl_], axis=2))  # [128,NDC,2,S]

    cosT = np.ascontiguousarray(np.asarray(cos, np.float32).T).astype(bf)
    sinT = np.ascontiguousarray(np.asarray(sin, np.float32).T).astype(bf)

    rmat = np.zeros((HD, HD), np.float32)
    half = HD // 2
    rmat[np.arange(half), np.arange(half) + half] = 1.0
    rmat[np.arange(half) + half, np.arange(half)] = -1.0
    rmat = rmat.astype(bf)

    # diagonal pair mask: keep when t_local (= i*128 + p) <= s_local
    lt = np.arange(128)[:, None]
    ls = np.arange(256)[None, :]
    masks = np.concatenate([(lt + 128 * i <= ls) for i in range(2)], axis=1)
    masks = np.ascontiguousarray(masks).astype(bf)     # [128, 512]

    scale = 1.0 / np.sqrt(np.float32(HD))
    Wq_ = np.asarray(Wq, np.float32) * scale
    Wk_ = np.asarray(Wk, np.float32)
    Wv_ = np.asarray(Wv, np.float32)
    Wo_ = np.asarray(Wo, np.float32)

    def chunked(w):  # [D, m] -> [128, NDC, m]
        m = w.shape[1]
        return np.ascontiguousarray(
            w.reshape(NDC, 128, m).transpose(1, 0, 2))

    in_maps = []
    for r in range(N_CORES):
        wqh_, wql_ = hilo(chunked(Wq_[:, r * QW:(r + 1) * QW]), WQS)
        wqb_ = np.ascontiguousarray(np.stack([wqh_, wql_], axis=2))
        wkh_, wkl_ = hilo(chunked(Wk_[:, r * HD:(r + 1) * HD]), WKS)
        wkb_ = np.ascontiguousarray(np.stack([wkh_, wkl_], axis=2))
        wvh_, wvl_ = hilo(chunked(Wv_[:, r * HD:(r + 1) * HD]), WVS)
        wvb_ = np.ascontiguousarray(np.stack([wvh_, wvl_], axis=2))
        wo_r = np.ascontiguousarray(
            Wo_[r * QW:(r + 1) * QW, :].reshape(HPC, 128, D)
            .transpose(1, 0, 2))
        woh_, wol_ = hilo(wo_r, WOS)
        wob_ = np.ascontiguousarray(np.stack([woh_, wol_], axis=1))
        in_maps.append({
            "xb": xb_, "wqb": wqb_, "wkb": wkb_, "wvb": wvb_, "wob": wob_,
            "cosT": cosT, "sinT": sinT, "rmat": rmat, "masks": masks,
        })
    return in_maps


def get_nc():
    if "nc" not in _CACHE:
        _CACHE["nc"] = _build()
    return _CACHE["nc"]


def kernel(x, mask, cos, sin, Wq, Wk, Wv, Wo):
    nc = get_nc()
    in_maps = _prep_inputs(x, cos, sin, Wq, Wk, Wv, Wo)
    res = run_bass_kernel_spmd(nc, in_maps, core_ids=list(range(N_CORES)))
    acc = np.zeros((S, D), np.float32)
    for r in range(N_CORES):
        acc += res.results[r]["out"].astype(np.float32)
    return acc[None]


if __name__ == "__main__":
    print("built:", get_nc() is not None)



# revision 86
# speedup vs baseline: 1.0140x; 1.0060x over previous
"""Grouped-query attention, tensor-parallel over heads across 8 TRN2 NeuronCores.

Problem (hardcoded): x[1,1024,4096] @ Wq/Wk/Wv -> RoPE -> causal GQA
(32 q heads, 8 kv groups, head_dim 128) -> out proj Wo -> [1,1024,4096].

Sharding: core r owns q heads 4r..4r+3 and kv group r (Wq/Wk/Wv column
shards, Wo row shard). Each core computes a full [1024,4096] partial of
the output projection; the host sums the 8 partials (the "all-reduce").

Device kernel (per core): the big GEMMs (Q/K/V projections, out-proj)
run in fp8e4 DoubleRow mode, which processes two 128-deep contraction
chunks per instruction at half the cycles/row of bf16.  Precision is
recovered with a 3-term hi/lo split quantization (x_hi@W_hi + x_lo@W_hi
+ x_hi@W_lo), where hi and lo shares one power-of-2 scale so all terms
accumulate in a single PSUM chain; measured end-to-end error matches
bf16.  The attention core (scores, exp, ctx) stays bf16 with 256-wide
s-blocks and causal tile skipping.  The softmax denominator is computed
off the tensor engine entirely: a DVE tree-sum of the P tiles, a Pool
partition_all_reduce, and a bf16 reciprocal.  The first s-block's
attention stages are pre-emitted into the second projection half's
instruction stream (scores matmuls fill DMA-paced slack; their exps run
on the then-idle Act engine), and output-projection tiles stream out
through partial-region DMAs so only a 512-column transfer trails the
final matmul.
"""

import numpy as np
import ml_dtypes

import concourse.bass as bass
import concourse.bacc as bacc
import concourse.mybir as mybir
import concourse.tile as tile
import concourse.bass_isa as bass_isa
from concourse.bass_utils import run_bass_kernel_spmd

S = 1024          # sequence length
D = 4096          # model dim
H = 32            # query heads (global)
G = 8             # kv groups (global)
HD = 128          # head dim
N_CORES = 8
HPC = H // N_CORES   # 4 query heads per core
QW = HPC * HD        # 512 q-proj cols per core
NDC = D // 128       # 32 contraction chunks
NP = NDC // 2        # 16 DoubleRow chunk pairs
BF = mybir.dt.bfloat16
F8 = mybir.dt.float8e4
F32 = mybir.dt.float32
DR = mybir.MatmulPerfMode.DoubleRow

# quantization scales (powers of 2; hi and lo share the scale so every
# 3-term matmul accumulates in one PSUM chain)
XS = 16.0
WQS = 8192.0        # applied to Wq/sqrt(HD)
WKS = 1024.0
WVS = 1024.0
WOS = 1024.0
CTXS = 16.0
EXP_SHIFT = -6.0    # exp(s - 6): keeps bf16 P comfortably in range
N_WARM = 38         # startup PE warm-up matmuls bridging the DMA fill

_CACHE = {}


def _build():
    nc = bacc.Bacc("TRN2", target_bir_lowering=False, debug=False,
                   num_devices=N_CORES)

    xb = nc.dram_tensor("xb", [128, NDC, 2, S], F8, kind="ExternalInput")
    wqb = nc.dram_tensor("wqb", [128, NDC, 2, QW], F8, kind="ExternalInput")
    wkb = nc.dram_tensor("wkb", [128, NDC, 2, HD], F8, kind="ExternalInput")
    wvb = nc.dram_tensor("wvb", [128, NDC, 2, HD], F8, kind="ExternalInput")
    wob = nc.dram_tensor("wob", [128, 2, HPC, D], F8, kind="ExternalInput")
    cosT = nc.dram_tensor("cosT", [HD, S], BF, kind="ExternalInput")
    sinT = nc.dram_tensor("sinT", [HD, S], BF, kind="ExternalInput")
    rmat = nc.dram_tensor("rmat", [HD, HD], BF, kind="ExternalInput")
    masks = nc.dram_tensor("masks", [128, 512], BF, kind="ExternalInput")
    out = nc.dram_tensor("out", [S, D], BF, kind="ExternalOutput")

    with tile.TileContext(nc) as tc:
        _emit(tc, nc, xb, wqb, wkb, wvb, wob,
              cosT, sinT, rmat, masks, out)
    nc.compile()
    return nc


def _emit(tc, nc, xb, wqb, wkb, wvb, wob,
          cosT, sinT, rmat, masks, out):
    import contextlib
    ctx = contextlib.ExitStack()
    with ctx:
        const = ctx.enter_context(tc.tile_pool(name="const", bufs=1))
        work = ctx.enter_context(tc.tile_pool(name="work", bufs=1))
        tmp = ctx.enter_context(tc.tile_pool(name="tmp", bufs=4))
        pt_pool = ctx.enter_context(tc.tile_pool(name="pt", bufs=8))
        outp = ctx.enter_context(tc.tile_pool(name="outp", bufs=2))
        ps = ctx.enter_context(tc.tile_pool(name="ps", bufs=8, space="PSUM"))

        # ---- DMA emission, ordered to pace the chunk-major PE stream ----
        rmat_sb = const.tile([HD, HD], BF, tag="rmat")
        # rmat memset first: it gates the PE warm-up start
        nc.vector.memset(rmat_sb[:], 0.5)
        ebias = const.tile([128, 1], F32, tag="ebias")
        nc.gpsimd.memset(ebias[:], EXP_SHIFT)

        wk_sb = const.tile([128, NDC, 2, HD], F8, tag="wk")   # dim2: hi/lo

        # PE warm-up bridge: junk matmuls over the startup DMA fill keep the
        # tensor engine's clock-ramp streak alive until real operands land.
        wps = ps.tile([128, 512], F32, tag="ps", name="wps")
        for wi in range(N_WARM):
            nc.tensor.matmul(wps[:, 0:HD], rmat_sb[:], rmat_sb[:],
                             start=(wi == 0), stop=(wi == N_WARM - 1))
        nc.vector.tensor_copy(rmat_sb[:], wps[:, 0:HD])
        nc.sync.dma_start(out=rmat_sb[:], in_=rmat.ap())

        gx4, gq4 = {}, {}
        for c in range(0, NDC, 4):
            gx4[c] = const.tile([128, 4, 2, S], F8, tag=f"x{c//4}", name=f"x{c//4}")
        gxh = {c: gx4[c - c % 4][:, c % 4:c % 4 + 2, 0, :] for c in range(0, NDC, 2)}
        gxl = {c: gx4[c - c % 4][:, c % 4:c % 4 + 2, 1, :] for c in range(0, NDC, 2)}
        gqh, gql = {}, {}
        # half-0 of x, wq, and wk interleaved in consumption order
        for c in range(0, NDC, 4):
            nc.sync.dma_start(out=wk_sb[:, c:c + 4, :, :],
                              in_=wkb.ap()[:, c:c + 4, :, :])
            g = const.tile([128, 4, 2, QW], F8, tag=f"q{c//4}", name=f"q{c//4}")
            gqh[c], gqh[c + 2] = g[:, 0:2, 0, :], g[:, 2:4, 0, :]
            gql[c], gql[c + 2] = g[:, 0:2, 1, :], g[:, 2:4, 1, :]
            if c == 0:
                # fine-grained early groups so the PE stream starts early
                for c2 in (0, 2):
                    nc.sync.dma_start(out=g[:, c2:c2 + 2, :, :],
                                      in_=wqb.ap()[:, c + c2:c + c2 + 2, :, :])
                    nc.sync.dma_start(
                        out=gx4[c][:, c2:c2 + 2, :, 0:512],
                        in_=xb.ap()[:, c + c2:c + c2 + 2, :, 0:512])
            else:
                for c2 in (0, 2):
                    nc.sync.dma_start(out=g[:, c2:c2 + 2, :, :],
                                      in_=wqb.ap()[:, c + c2:c + c2 + 2, :, :])
                    nc.sync.dma_start(
                        out=gx4[c][:, c2:c2 + 2, :, 0:512],
                        in_=xb.ap()[:, c + c2:c + c2 + 2, :, 0:512])
        cos_sb = const.tile([HD, S], BF, tag="cos")
        nc.sync.dma_start(out=cos_sb[:], in_=cosT.ap())
        sin_sb = const.tile([HD, S], BF, tag="sin")
        nc.sync.dma_start(out=sin_sb[:], in_=sinT.ap())
        # half-1 of x
        for c in range(0, NDC, 4):
            nc.sync.dma_start(out=gx4[c][:, :, :, 512:S],
                              in_=xb.ap()[:, c:c + 4, :, 512:S])
        wv_sb = const.tile([128, NDC, 2, HD], F8, tag="wv")
        nc.sync.dma_start(out=wv_sb[:], in_=wvb.ap())
        mask_sb = const.tile([128, 512], BF, tag="mask")
        nc.sync.dma_start(out=mask_sb[:], in_=masks.ap())
        wo_sb = const.tile([128, 2, HPC, D], F8, tag="wo")    # dim1: hi/lo
        for n in range(2):
            sl = slice(n * 2048, (n + 1) * 2048)
            nc.sync.dma_start(out=wo_sb[:, :, :, sl], in_=wob.ap()[:, :, :, sl])

        # persistent activations
        khat = work.tile([HD, S], BF, tag="khat")
        qhat = [work.tile([HD, S], BF, tag=f"qhat{h}", name=f"qhat{h}")
                for h in range(HPC)]
        v_sb = [work.tile([128, HD], BF, tag=f"v{i}", name=f"v{i}")
                for i in range(8)]
        ctx_hi = [work.tile([128, 2, S], F8, tag=f"cth{u}", name=f"cth{u}")
                  for u in range(2)]
        ctx_lo = [work.tile([128, 2, S], F8, tag=f"ctl{u}", name=f"ctl{u}")
                  for u in range(2)]

        # ---- K+Q projections: chunk-major across 5 chains per s-quarter ----
        # Per chunk pair, all five tensors advance their 3-term DoubleRow
        # chains, so the PE stream follows the x/wq DMA arrival order.
        # RoPE for each finished s-half is queued and its PE/Act/DVE work is
        # injected into later quarters' streams (and the v-projection).
        TENS = [("k", khat, lambda c: wk_sb[:, c:c + 2, 0, :],
                 lambda c: wk_sb[:, c:c + 2, 1, :], 1.0 / (XS * WKS))]
        for h in range(HPC):
            hsl = slice(h * HD, (h + 1) * HD)
            TENS.append((f"q{h}", qhat[h],
                         lambda c, s=hsl: gqh[c][:, :, s],
                         lambda c, s=hsl: gql[c][:, :, s], 1.0 / (XS * WQS)))
        raws = {ti: work.tile([HD, S], BF, tag=f"raw{ti}", name=f"raw{ti}")
                for ti in range(5)}

        pend = []   # queued injection closures (ropes, early scores)

        def inject_rope():
            if pend:
                pend.pop(0)()

        def rope_half(ti, half):
            name, dst, _, _, _ = TENS[ti]
            sl = slice(half * 512, (half + 1) * 512)
            t1 = tmp.tile([HD, 512], BF, tag="rope_t1", name="rope_t1", bufs=1)
            nc.vector.tensor_mul(t1[:], raws[ti][:, sl], cos_sb[:, sl])
            rq = ps.tile([HD, 512], F32, tag="ps", name="rq")
            nc.tensor.matmul(rq[:], rmat_sb[:], raws[ti][:, sl],
                             start=True, stop=True)
            t2 = tmp.tile([HD, 512], BF, tag="rope_t2", name="rope_t2", bufs=1)
            nc.vector.tensor_mul(t2[:], rq[:], sin_sb[:, sl])
            nc.vector.tensor_add(dst[:, sl], t1[:], t2[:])

        def v_terms(vpsum, i, p):
            c = 2 * p
            tsl = slice(i * 128, (i + 1) * 128)
            nc.tensor.matmul(vpsum[:], gxh[c][:, :, tsl], wv_sb[:, c:c + 2, 0, :],
                             start=(p == 0), stop=False,
                             perf_mode=DR, skip_group_check=True)
            nc.tensor.matmul(vpsum[:], gxh[c][:, :, tsl], wv_sb[:, c:c + 2, 1, :],
                             start=False, stop=False,
                             perf_mode=DR, skip_group_check=True)
            nc.tensor.matmul(vpsum[:], gxl[c][:, :, tsl], wv_sb[:, c:c + 2, 0, :],
                             start=False, stop=(p == NP - 1),
                             perf_mode=DR, skip_group_check=True)

        pre_pts = {}   # stages pre-emitted into the QK stream
        def emit_scores(h, b):
            ssl = slice(b * 256, (b + 1) * 256)
            pts = []
            for tp in range(b + 1):
                st = ps.tile([128, 512], F32, tag="ps", name="st")
                for i in range(2):
                    t0 = (2 * tp + i) * 128
                    nc.tensor.matmul(st[:, i * 256:(i + 1) * 256],
                                     khat[:, t0:t0 + 128], qhat[h][:, ssl],
                                     start=True, stop=True)
                pt = pt_pool.tile([128, 512], BF, tag="pt", name="pt")
                nc.scalar.activation(pt[:], st[:],
                                     mybir.ActivationFunctionType.Exp,
                                     bias=ebias[:])
                if tp == b:
                    nc.vector.tensor_mul(pt[:], pt[:], mask_sb[:])
                pts.append(pt)
            # Denominator off the PE: DVE tree-sum of P tiles, Pool
            # partition-all-reduce, DVE half-merge + reciprocal. All emitted
            # here so the result has a full pipeline stage of slack.
            if b == 0:
                acc = pts[0]
            else:
                acc = tmp.tile([128, 512], BF, tag="dacc", name="dacc", bufs=2)
                if b == 1:
                    nc.vector.tensor_add(acc[:], pts[0][:], pts[1][:])
                elif b == 2:
                    t01 = tmp.tile([128, 512], BF, tag="dacc2", name="dacc2",
                                   bufs=1)
                    nc.vector.tensor_add(t01[:], pts[0][:], pts[1][:])
                    nc.vector.tensor_add(acc[:], t01[:], pts[2][:])
                else:
                    t01 = tmp.tile([128, 512], BF, tag="dacc2", name="dacc2",
                                   bufs=1)
                    t23 = tmp.tile([128, 512], BF, tag="dacc3", name="dacc3",
                                   bufs=1)
                    nc.vector.tensor_add(t01[:], pts[0][:], pts[1][:])
                    nc.vector.tensor_add(t23[:], pts[2][:], pts[3][:])
                    nc.vector.tensor_add(acc[:], t01[:], t23[:])
            denb = tmp.tile([128, 512], BF, tag="denb", name="denb", bufs=2)
            nc.gpsimd.partition_all_reduce(denb[:], acc[:], 128,
                                           bass_isa.ReduceOp.add)
            rec = tmp.tile([128, 256], BF, tag="rec", name="rec", bufs=5)
            with nc.allow_low_precision(reason="den scale, 0.2% rel is fine"):
                nc.vector.tensor_add(rec[:], denb[:, 0:256],
                                     denb[:, 256:512])
                nc.vector.reciprocal(rec[:], rec[:])
            return pts, rec


        v_first = []   # first v-chains, interleaved with half-1 descales

        for half in (0, 1):
            hoff = half * 512
            chains = [ps.tile([128, 512], F32, tag="ps", name=f"ch{ti}")
                      for ti in range(5)]
            for p in range(NP):
                c = 2 * p
                for term in range(3):
                    for ti, (_, _, whi, wlo, _) in enumerate(TENS):
                        pp = chains[ti]
                        w = whi(c) if term != 1 else wlo(c)
                        for q in range(2):
                            ssl = slice(hoff + q * 256, hoff + (q + 1) * 256)
                            osl = slice(q * 256, (q + 1) * 256)
                            xop = gxl[c] if term == 2 else gxh[c]
                            nc.tensor.matmul(
                                pp[:, osl], w, xop[:, :, ssl],
                                start=(p == 0 and term == 0 and q == 0),
                                stop=(p == NP - 1 and term == 2 and q == 1),
                                perf_mode=DR, skip_group_check=True)
                if p in (2, 4, 6, 8, 10, 12, 14):
                    inject_rope()
            if half == 0:
                for ti, (_, _, _, _, descale) in enumerate(TENS):
                    nc.scalar.activation(raws[ti][:, 0:512], chains[ti][:],
                                         mybir.ActivationFunctionType.Copy,
                                         scale=descale)
                def early_scores(h):
                    pre_pts[(h, 0)] = emit_scores(h, 0)
                pend.extend([
                    lambda: rope_half(0, 0),    # k
                    lambda: rope_half(1, 0),    # q0
                    lambda: rope_half(2, 0),    # q1
                    lambda: early_scores(0),
                    lambda: rope_half(3, 0),    # q2
                    lambda: early_scores(1),
                    lambda: rope_half(4, 0),    # q3
                ])
            else:
                # defer the 5 serial ACT descales until after the first
                # attention stage's exps are queued (ACT priority)
                def half1_tail(chains=chains):
                    for ti, (_, _, _, _, descale) in enumerate(TENS):
                        nc.scalar.activation(
                            raws[ti][:, 512:S], chains[ti][:],
                            mybir.ActivationFunctionType.Copy, scale=descale)
                        pend.append(lambda t=ti: rope_half(t, 1))
                v_first.append(half1_tail)

        # ---- V projection: emitted as PE filler inside early attention ----
        def v_chain(i):
            tsl = slice(i * 128, (i + 1) * 128)
            vp = ps.tile([128, HD], F32, tag="ps", name="vp")
            for p in range(NP):
                c = 2 * p
                nc.tensor.matmul(vp[:], gxh[c][:, :, tsl], wv_sb[:, c:c + 2, 0, :],
                                 start=(p == 0), stop=False, perf_mode=DR)
            for p in range(NP):
                c = 2 * p
                nc.tensor.matmul(vp[:], gxh[c][:, :, tsl], wv_sb[:, c:c + 2, 1, :],
                                 start=False, stop=False, perf_mode=DR)
            for p in range(NP):
                c = 2 * p
                nc.tensor.matmul(vp[:], gxl[c][:, :, tsl], wv_sb[:, c:c + 2, 0, :],
                                 start=False, stop=(p == NP - 1), perf_mode=DR)
            nc.scalar.activation(v_sb[i][:], vp[:],
                                 mybir.ActivationFunctionType.Copy,
                                 scale=1.0 / (XS * WVS))
            inject_rope()

        pre_pts[(2, 0)] = emit_scores(2, 0)
        v_first[0]()          # the deferred half-1 descales
        v_chain(0)
        pre_pts[(3, 0)] = emit_scores(3, 0)
        v_chain(1)
        vq = list(range(2, 8))

        def emit_denctx(h, b, pts, rec):
            ssl = slice(b * 256, (b + 1) * 256)
            cx = ps.tile([HD, 256], F32, tag="ps", name="cx")
            n_mm = 2 * (b + 1)
            k = 0
            for tp, pt in enumerate(pts):
                for i in range(2):
                    if tp == b and i == 1:
                        # diagonal pair: tokens 128-255 only reach s>=128
                        nc.tensor.matmul(cx[:, 128:256], v_sb[2 * tp + i][:],
                                         pt[:, 384:512],
                                         start=False, stop=(k == n_mm - 1))
                    else:
                        nc.tensor.matmul(cx[:], v_sb[2 * tp + i][:],
                                         pt[:, i * 256:(i + 1) * 256],
                                         start=(k == 0), stop=(k == n_mm - 1))
                    k += 1
            ctxn = tmp.tile([HD, 256], F32, tag="ctxn", name="ctxn", bufs=2)
            nc.vector.scalar_tensor_tensor(
                ctxn[:], cx[:], CTXS, rec[:],
                op0=mybir.AluOpType.mult, op1=mybir.AluOpType.mult)
            u, par = divmod(h, 2)
            nc.vector.tensor_copy(ctx_hi[u][:, par, ssl], ctxn[:])
            nc.vector.tensor_sub(ctx_lo[u][:, par, ssl], ctxn[:],
                                 ctx_hi[u][:, par, ssl])

        descale = 1.0 / (CTXS * WOS)

        def emit_outproj(t8, last=False):
            tsl = slice(t8 * 128, (t8 + 1) * 128)
            for n2 in range(2):
                ot = outp.tile([128, 2048], BF, tag="ot", name="ot")
                # partial-region DMAs [1024|512|512]; the very last tile
                # uses [1536|512] so only one HWDGE generation trails the
                # final matmul
                if last and n2 == 1:
                    dma_after = {1: (0, 1024), 3: (1024, 1024)}
                else:
                    dma_after = {1: (0, 1024), 2: (1024, 512),
                                 3: (1536, 512)}
                for half2 in range(4):
                    op = ps.tile([128, 512], F32, tag="ps", name="op")
                    for sub2 in range(2):
                        n = 8 * n2 + 2 * half2 + sub2
                        nsl = slice(n * 256, (n + 1) * 256)
                        osl = slice(sub2 * 256, (sub2 + 1) * 256)
                        k = 0
                        for u in range(2):
                            for chi, whi in ((ctx_hi, 0), (ctx_lo, 0),
                                             (ctx_hi, 1)):
                                nc.tensor.matmul(
                                    op[:, osl], chi[u][:, :, tsl],
                                    wo_sb[:, whi, 2 * u:2 * u + 2, nsl],
                                    start=(k == 0 and sub2 == 0),
                                    stop=(k == 5 and sub2 == 1),
                                    perf_mode=DR, skip_group_check=True)
                                k += 1
                    osl2 = slice(half2 * 512, (half2 + 1) * 512)
                    if half2 == 2:
                        nc.vector.tensor_scalar_mul(ot[:, osl2], op[:],
                                                    descale)
                    else:
                        nc.scalar.activation(
                            ot[:, osl2], op[:],
                            mybir.ActivationFunctionType.Copy,
                            scale=descale)
                    if half2 in dma_after:
                        o0, ow = dma_after[half2]
                        nc.sync.dma_start(
                            out=out.ap()[tsl, n2 * 2048 + o0:
                                         n2 * 2048 + o0 + ow],
                            in_=ot[:, o0:o0 + ow])

        stages = [(h, b) for b in range(4) for h in range(HPC)]
        prev = None
        outq = []
        cooldown = 0
        for si, hb in enumerate(stages):
            if hb in pre_pts:
                pts, dacc = pre_pts.pop(hb)
            else:
                pts, dacc = emit_scores(*hb)
            todo = None
            if cooldown:
                cooldown -= 1
            elif outq:
                todo = outq.pop(0)
            if prev is not None:
                (ph, pb), ppts, pdacc = prev
                emit_denctx(ph, pb, ppts, pdacc)
                if ph == HPC - 1:
                    outq.extend([2 * pb + 1, 2 * pb])
                    cooldown = 1
            if todo is not None:
                emit_outproj(todo)
            inject_rope()
            for _ in range(2 if si < 3 else 1):
                if vq:
                    v_chain(vq.pop(0))
            if si + 1 < len(stages):
                nxt = stages[si + 1]
                if nxt not in pre_pts:
                    pre_pts[nxt] = emit_scores(*nxt)
            prev = (hb, pts, dacc)
        (ph, pb), ppts, pdacc = prev
        emit_denctx(ph, pb, ppts, pdacc)
        outq.extend([2 * pb + 1, 2 * pb])
        for qi, t8 in enumerate(outq):
            emit_outproj(t8, last=(qi == len(outq) - 1))


def _prep_inputs(x, cos, sin, Wq, Wk, Wv, Wo):
    """Host-side shard + hi/lo fp8 quantization. Returns per-core inputs."""
    bf = ml_dtypes.bfloat16
    f8 = ml_dtypes.float8_e4m3

    def hilo(a, s):
        hi = np.asarray(a * s, np.float32).astype(f8)
        lo = (np.asarray(a * s, np.float32) - hi.astype(np.float32)).astype(f8)
        return hi, lo

    x2 = np.asarray(x, np.float32).reshape(S, D)
    xTh = np.ascontiguousarray(x2.T).reshape(NDC, 128, S).transpose(1, 0, 2)
    xh_, xl_ = hilo(np.ascontiguousarray(xTh), XS)
    xb_ = np.ascontiguousarray(np.stack([xh_, xl_], axis=2))  # [128,NDC,2,S]

    cosT = np.ascontiguousarray(np.asarray(cos, np.float32).T).astype(bf)
    sinT = np.ascontiguousarray(np.asarray(sin, np.float32).T).astype(bf)

    rmat = np.zeros((HD, HD), np.float32)
    half = HD // 2
    rmat[np.arange(half), np.arange(half) + half] = 1.0
    rmat[np.arange(half) + half, np.arange(half)] = -1.0
    rmat = rmat.astype(bf)

    # diagonal pair mask: keep when t_local (= i*128 + p) <= s_local
    lt = np.arange(128)[:, None]
    ls = np.arange(256)[None, :]
    masks = np.concatenate([(lt + 128 * i <= ls) for i in range(2)], axis=1)
    masks = np.ascontiguousarray(masks).astype(bf)     # [128, 512]

    scale = 1.0 / np.sqrt(np.float32(HD))
    Wq_ = np.asarray(Wq, np.float32) * scale
    Wk_ = np.asarray(Wk, np.float32)
    Wv_ = np.asarray(Wv, np.float32)
    Wo_ = np.asarray(Wo, np.float32)

    def chunked(w):  # [D, m] -> [128, NDC, m]
        m = w.shape[1]
        return np.ascontiguousarray(
            w.reshape(NDC, 128, m).transpose(1, 0, 2))

    in_maps = []
    for r in range(N_CORES):
        wqh_, wql_ = hilo(chunked(Wq_[:, r * QW:(r + 1) * QW]), WQS)
        wqb_ = np.ascontiguousarray(np.stack([wqh_, wql_], axis=2))
        wkh_, wkl_ = hilo(chunked(Wk_[:, r * HD:(r + 1) * HD]), WKS)
        wkb_ = np.ascontiguousarray(np.stack([wkh_, wkl_], axis=2))
        wvh_, wvl_ = hilo(chunked(Wv_[:, r * HD:(r + 1) * HD]), WVS)
        wvb_ = np.ascontiguousarray(np.stack([wvh_, wvl_], axis=2))
        wo_r = np.ascontiguousarray(
            Wo_[r * QW:(r + 1) * QW, :].reshape(HPC, 128, D)
            .transpose(1, 0, 2))
        woh_, wol_ = hilo(wo_r, WOS)
        wob_ = np.ascontiguousarray(np.stack([woh_, wol_], axis=1))
        in_maps.append({
            "xb": xb_, "wqb": wqb_, "wkb": wkb_, "wvb": wvb_, "wob": wob_,
            "cosT": cosT, "sinT": sinT, "rmat": rmat, "masks": masks,
        })
    return in_maps


def get_nc():
    if "nc" not in _CACHE:
        _CACHE["nc"] = _build()
    return _CACHE["nc"]


def kernel(x, mask, cos, sin, Wq, Wk, Wv, Wo):
    nc = get_nc()
    in_maps = _prep_inputs(x, cos, sin, Wq, Wk, Wv, Wo)
    res = run_bass_kernel_spmd(nc, in_maps, core_ids=list(range(N_CORES)))
    acc = np.zeros((S, D), np.float32)
    for r in range(N_CORES):
        acc += res.results[r]["out"].astype(np.float32)
    return acc[None]


if __name__ == "__main__":
    print("built:", get_nc() is not None)

